# revision 10
# baseline (speedup 1.0000x reference)
"""Trainium2 Bass kernel for nn_BlocksCore (RIMs BlocksCore step).

Strategy: data-parallel over batch B=2048 across 8 NeuronCores (256 rows
each). All parameters replicated. Per-core computation:

  1. input attention (f32): k1 = inp@wk1, v1 = inp@wv1, q_n = hx_n@wq_n,
     s1[b,n] = q_n.k1 / 8 (zero-slot score is exactly 0, so softmax over
     [0, s1] collapses to sigmoid);  inp_flat[b, n*256+j] = sig(s1)[b,n]*v1[b,j]
  2. top-k mask: keep the 4 blocks with largest s1 (drop the 4 that attend
     most to the null slot), per row
  3. LSTM cell (bf16 matmuls): gates = [inp_flat|hx] @ [w_ih|w_hh]^T + b
  4. communication attention among the 8 blocks (4 heads, dk=dv=32),
     gated residual: hx_new = h + sigmoid(c@gw+gb)*tanh(c@fw+fb)
  5. masked update of hx/cx.

Layout: batch-major ([batch_p, feat]) for elementwise work; matmuls take
pre-transposed (feature-major) activations as stationary operands. Weights
are pre-transposed/cast on the host. Outputs are batch-major, so the host
just concatenates the 8 shards.
"""

import json
import os

import numpy as np
import ml_dtypes

BF16 = ml_dtypes.bfloat16

B = 2048
NCORES = 8
BSH = B // NCORES          # 256 batch rows per core
NINP = 1024
NHID = 2048
NB = 8                     # blocks
BS = 256                   # block size (NHID / NB)
DKI = 64                   # input-attention d_k
GATES = 4 * NHID           # 8192
KX = 2 * NHID              # LSTM contraction: [inp_flat(2048) | hx(2048)]

_CACHE = {}
last_exec_time_ns = None
last_results = None

# ---------------------------------------------------------------------------
# BIR post-fix: this toolchain's core_v3 codegen supports only one sync-wait
# per CTRL-class instruction (Drain/NoOp/branch). Tile's final drain can carry
# several; hoist extras onto single-wait EventSemaphore instructions.
# ---------------------------------------------------------------------------
# applies to every opcode in this build, so split waits on all of them


def _fix_bir_json(bir_bytes: bytes) -> bytes:
    bir = json.loads(bir_bytes)
    for fn in bir.get("functions", []):
        for blk in fn.get("blocks", []):
            out = []
            for ins in blk.get("instructions", []):
                si = ins.get("sync_info") or {}
                waits = si.get("on_wait") or []
                if len(waits) > 1:
                    for j, w in enumerate(waits[:-1]):
                        out.append({
                            "name": f"{ins['name']}-w{j}",
                            "engine": ins["engine"],
                            "opcode": "EventSemaphore",
                            "ins": [],
                            "outs": [],
                            "sync_info": {"on_update": [], "on_wait": [w]},
                        })
                    si = dict(si)
                    si["on_wait"] = [waits[-1]]
                    ins = dict(ins)
                    ins["sync_info"] = si
                out.append(ins)
            blk["instructions"] = out
    return json.dumps(bir).encode()


def _install_bir_fix(nc):
    orig = nc.to_json_bytes

    def patched(*a, **k):
        return _fix_bir_json(orig(*a, **k))

    nc.to_json_bytes = patched


# ---------------------------------------------------------------------------
# Device kernel
# ---------------------------------------------------------------------------

def _build():
    import concourse.bass as bass
    import concourse.tile as tile
    from concourse import mybir

    f32 = mybir.dt.float32
    bf16 = mybir.dt.bfloat16
    OP = mybir.AluOpType
    AF = mybir.ActivationFunctionType
    AX = mybir.AxisListType

    nc = bass.Bass()

    # ---- I/O ------------------------------------------------------------
    inpT = nc.declare_dram_parameter("inpT", [128, 8, BSH], f32, isOutput=False)
    hxT_f = nc.declare_dram_parameter("hxT_f", [128, 16, BSH], f32, isOutput=False)
    hxT_b = nc.declare_dram_parameter("hxT_b", [128, 16, BSH], bf16, isOutput=False)
    hx_bm = nc.declare_dram_parameter("hx_bm", [BSH, NHID], f32, isOutput=False)
    cx_bm = nc.declare_dram_parameter("cx_bm", [BSH, NHID], f32, isOutput=False)
    wq = nc.declare_dram_parameter("wq", [128, 2, NB, DKI], f32, isOutput=False)
    wk1 = nc.declare_dram_parameter("wk1", [128, 8, DKI], f32, isOutput=False)
    wv1 = nc.declare_dram_parameter("wv1", [128, 8, BS], f32, isOutput=False)
    wcatT = nc.declare_dram_parameter("wcatT", [KX, GATES], bf16, isOutput=False)
    biasc = nc.declare_dram_parameter("biasc", [1, GATES], bf16, isOutput=False)
    wqc = nc.declare_dram_parameter("wqc", [128, 2, NB, 128], bf16, isOutput=False)
    wkc = nc.declare_dram_parameter("wkc", [128, 2, NB, 128], bf16, isOutput=False)
    wvc = nc.declare_dram_parameter("wvc", [128, 2, NB, 128], bf16, isOutput=False)
    fcw = nc.declare_dram_parameter("fcw", [128, BS], bf16, isOutput=False)
    gw = nc.declare_dram_parameter("gw", [128, BS], bf16, isOutput=False)
    fcb = nc.declare_dram_parameter("fcb", [1, BS], bf16, isOutput=False)
    gb = nc.declare_dram_parameter("gb", [1, BS], bf16, isOutput=False)
    hx_out = nc.declare_dram_parameter("hx_out", [BSH, NHID], f32, isOutput=True)
    cx_out = nc.declare_dram_parameter("cx_out", [BSH, NHID], f32, isOutput=True)
    mask_out = nc.declare_dram_parameter("mask_out", [BSH, NHID], f32, isOutput=True)

    # ---- inline constants ----------------------------------------------
    ident_np = np.eye(128, dtype=BF16)
    # score-placement selector: for query block q, out row m = h*8+q gets the
    # head-h sum of a [128]-feature product vector (d -> h = d//32)
    hq_np = np.zeros((128, NB, 32), dtype=BF16)
    for d in range(128):
        for q in range(NB):
            hq_np[d, q, (d // 32) * 8 + q] = 1
    # head expander: for query block q, out feature m (=h*32+d) reads score
    # row r = (m//32)*8 + q
    e32_np = np.zeros((32, NB, 128), dtype=BF16)
    for m in range(128):
        for q in range(NB):
            e32_np[(m // 32) * 8 + q, q, m] = 1
    identb = nc.inline_tensor(ident_np, "identb")
    hqc = nc.inline_tensor(hq_np, "hqc")
    e32b = nc.inline_tensor(e32_np, "e32b")
    e32f = nc.inline_tensor(e32_np.astype(np.float32), "e32f")
    ones1c = nc.inline_tensor(np.ones((1, 128), dtype=BF16), "ones1c")

    with tile.TileContext(nc) as tc:
        with tc.tile_pool(name="cp", bufs=1) as cp, \
             tc.tile_pool(name="pp", bufs=1) as pp:
            # constants to SBUF
            identb_sb = cp.tile([128, 128], bf16)
            nc.sync.dma_start(out=identb_sb[:], in_=identb[:])
            hq_sb = cp.tile([128, NB, 32], bf16)
            nc.sync.dma_start(out=hq_sb[:], in_=hqc[:])
            e32b_sb = cp.tile([32, NB, 128], bf16)
            nc.sync.dma_start(out=e32b_sb[:], in_=e32b[:])
            e32f_sb = cp.tile([32, NB, 128], f32)
            nc.sync.dma_start(out=e32f_sb[:], in_=e32f[:])
            ones1_sb = cp.tile([1, 128], bf16)
            nc.sync.dma_start(out=ones1_sb[:], in_=ones1c[:])
            fcw_sb = cp.tile([128, BS], bf16)
            nc.sync.dma_start(out=fcw_sb[:], in_=fcw[:])
            gw_sb = cp.tile([128, BS], bf16)
            nc.sync.dma_start(out=gw_sb[:], in_=gw[:])
            fcb_sb = cp.tile([1, BS], bf16)
            nc.sync.dma_start(out=fcb_sb[:], in_=fcb[:])
            gb_sb = cp.tile([1, BS], bf16)
            nc.sync.dma_start(out=gb_sb[:], in_=gb[:])
            ones256_sb = cp.tile([128, BS], f32)
            nc.vector.memset(ones256_sb[:], 1.0)

            # persistent inputs / intermediates
            hxTb_sb = pp.tile([128, 16, BSH], bf16)
            nc.gpsimd.dma_start(out=hxTb_sb[:], in_=hxT_b[:])
            hx_sb = [pp.tile([128, NHID], f32, tag=f"hx{bt}", name=f"hx{bt}") for bt in range(2)]
            cx_sb = [pp.tile([128, NHID], f32, tag=f"cx{bt}", name=f"cx{bt}") for bt in range(2)]
            for bt in range(2):
                nc.gpsimd.dma_start(out=hx_sb[bt][:], in_=hx_bm[bt * 128:(bt + 1) * 128, :])
                nc.gpsimd.dma_start(out=cx_sb[bt][:], in_=cx_bm[bt * 128:(bt + 1) * 128, :])

            xt_sb = pp.tile([128, 16, 2, 128], bf16)      # inp_flat^T tiles
            hnew_sb = [pp.tile([128, NHID], f32, tag=f"hn{bt}", name=f"hn{bt}") for bt in range(2)]
            cnew_sb = [pp.tile([128, NHID], f32, tag=f"cn{bt}", name=f"cn{bt}") for bt in range(2)]
            hnewT_sb = pp.tile([128, 16, BSH], bf16)
            mask_sb = [pp.tile([128, NB], f32, tag=f"mk{bt}", name=f"mk{bt}") for bt in range(2)]
            sig_sb = [pp.tile([128, NB], f32, tag=f"sg{bt}", name=f"sg{bt}") for bt in range(2)]

            # ============================ phase A ========================
            with tc.tile_pool(name="pa", bufs=1) as pa, \
                 tc.tile_pool(name="pa2", bufs=2) as pa2, \
                 tc.tile_pool(name="paps", bufs=2, space="PSUM") as paps, \
                 tc.tile_pool(name="patp", bufs=2, space="PSUM") as patp:
                inpT_sb = pa.tile([128, 8, BSH], f32)
                nc.gpsimd.dma_start(out=inpT_sb[:], in_=inpT[:])
                hxTf_sb = pa.tile([128, 16, BSH], f32)
                nc.gpsimd.dma_start(out=hxTf_sb[:], in_=hxT_f[:])
                wk1_sb = pa.tile([128, 8, DKI], f32)
                nc.sync.dma_start(out=wk1_sb[:], in_=wk1[:])
                wv1_sb = pa.tile([128, 8, BS], f32)
                nc.sync.dma_start(out=wv1_sb[:], in_=wv1[:])
                wq_sb = pa.tile([128, 2, NB, DKI], f32)
                nc.sync.dma_start(out=wq_sb[:], in_=wq[:])

                for bt in range(2):
                    bsl = slice(bt * 128, (bt + 1) * 128)
                    k1_ps = paps.tile([128, DKI], f32, tag="k1")
                    for k in range(8):
                        nc.tensor.matmul(k1_ps[:], inpT_sb[:, k, bsl], wk1_sb[:, k, :],
                                         start=(k == 0), stop=(k == 7))
                    k1s = pa2.tile([128, DKI], f32, tag="k1s")
                    nc.vector.tensor_copy(k1s[:], k1_ps[:])

                    v1_ps = paps.tile([128, BS], f32, tag="v1")
                    for k in range(8):
                        nc.tensor.matmul(v1_ps[:], inpT_sb[:, k, bsl], wv1_sb[:, k, :],
                                         start=(k == 0), stop=(k == 7))
                    v1s = pa2.tile([128, BS], f32, tag="v1s")
                    nc.vector.tensor_copy(v1s[:], v1_ps[:])

                    q_ps = paps.tile([128, NB, DKI], f32, tag="q")
                    for n in range(NB):
                        for s in range(2):
                            nc.tensor.matmul(q_ps[:, n, :],
                                             hxTf_sb[:, 2 * n + s, bsl],
                                             wq_sb[:, s, n, :],
                                             start=(s == 0), stop=(s == 1))
                    prod = pa2.tile([128, NB, DKI], f32, tag="prod")
                    for n in range(NB):
                        nc.vector.tensor_tensor(prod[:, n, :], q_ps[:, n, :], k1s[:], OP.mult)
                    s1 = pa2.tile([128, NB], f32, tag="s1")
                    nc.vector.reduce_sum(s1[:], prod[:], axis=AX.X)
                    nc.scalar.activation(sig_sb[bt][:], s1[:], AF.Sigmoid, scale=0.125)

                    # top-4 mask: keep blocks whose s1 is among the 4 largest
                    cnt = pa2.tile([128, NB], f32, tag="cnt")
                    tmp = pa2.tile([128, NB], f32, tag="tmp")
                    for n in range(NB):
                        nc.vector.tensor_single_scalar(tmp[:], s1[:], s1[:, n:n + 1], OP.is_gt)
                        nc.vector.reduce_sum(cnt[:, n:n + 1], tmp[:], axis=AX.X)
                    nc.vector.tensor_single_scalar(mask_sb[bt][:], cnt[:], 4.0, OP.is_lt)

                    # inp_flat (batch-major, bf16) then transpose to xt tiles
                    ifl = pa2.tile([128, NB, BS], bf16, tag="ifl")
                    for n in range(NB):
                        nc.vector.tensor_single_scalar(ifl[:, n, :], v1s[:],
                                                       sig_sb[bt][:, n:n + 1], OP.mult)
                    for ft in range(16):
                        tp = patp.tile([128, 128], bf16, tag="tp")
                        nc.tensor.transpose(tp[:], ifl[:, ft // 2, (ft % 2) * 128:(ft % 2) * 128 + 128],
                                            identb_sb[:])
                        nc.scalar.copy(xt_sb[:, ft, bt, :], tp[:])

            # ============================ phase B ========================
            # wcatT/biasc columns are host-permuted: 512-wide block j holds
            # gate type j%4 (0=i,1=f,2=g,3=o) for hidden chunk j//4. Each odd
            # ncp completes a hidden chunk, so gate activations are computed
            # and released incrementally.
            with tc.tile_pool(name="pw", bufs=3) as pw, \
                 tc.tile_pool(name="pact", bufs=2) as pact, \
                 tc.tile_pool(name="pb2", bufs=2) as pb2, \
                 tc.tile_pool(name="pbps", bufs=1, space="PSUM") as pbps, \
                 tc.tile_pool(name="pbtp", bufs=2, space="PSUM") as pbtp:
                act_cur = {}
                for ncp in range(8):
                    g = {}
                    for bt in range(2):
                        for h in range(2):
                            g[bt, h] = pbps.tile([128, 512], f32, tag=f"g{bt}{h}", name=f"g{bt}{h}")
                    for k in range(32):
                        w = pw.tile([128, 1024], bf16)
                        eng = nc.sync if (k % 2 == 0) else nc.gpsimd
                        eng.dma_start(out=w[:],
                                      in_=wcatT[k * 128:(k + 1) * 128,
                                                ncp * 1024:(ncp + 1) * 1024])
                        for bt in range(2):
                            if k < 16:
                                lhsT = xt_sb[:, k, bt, :]
                            else:
                                lhsT = hxTb_sb[:, k - 16, bt * 128:(bt + 1) * 128]
                            for h in range(2):
                                nc.tensor.matmul(g[bt, h][:], lhsT, w[:, h * 512:(h + 1) * 512],
                                                 start=(k == 0), stop=False)
                    bsl_t = pb2.tile([1, 1024], bf16, tag="biasc", name="biascsl")
                    nc.sync.dma_start(out=bsl_t[:], in_=biasc[:, ncp * 1024:(ncp + 1) * 1024])
                    for bt in range(2):
                        for h in range(2):
                            jblk = 2 * ncp + h
                            gt_, t = jblk % 4, jblk // 4
                            nc.tensor.matmul(g[bt, h][:], ones1_sb[:],
                                             bsl_t[:, h * 512:(h + 1) * 512],
                                             start=False, stop=True)
                            a = pact.tile([128, 512], f32, tag=f"a{gt_}{bt}", name=f"a{gt_}{bt}")
                            func = AF.Tanh if gt_ == 2 else AF.Sigmoid
                            nc.scalar.activation(a[:], g[bt, h][:], func)
                            act_cur[gt_, bt] = a
                    if ncp % 2 == 1:
                        t = ncp // 2
                        sl = slice(t * 512, (t + 1) * 512)
                        for bt in range(2):
                            t1 = pb2.tile([128, 512], f32, tag="t1", name="t1")
                            nc.vector.tensor_tensor(t1[:], act_cur[1, bt][:],
                                                    cx_sb[bt][:, sl], OP.mult)
                            t2 = pb2.tile([128, 512], f32, tag="t2", name="t2")
                            nc.vector.tensor_tensor(t2[:], act_cur[0, bt][:],
                                                    act_cur[2, bt][:], OP.mult)
                            nc.vector.tensor_tensor(cnew_sb[bt][:, sl], t1[:], t2[:], OP.add)
                            t3 = pb2.tile([128, 512], f32, tag="t3", name="t3")
                            nc.scalar.activation(t3[:], cnew_sb[bt][:, sl], AF.Tanh)
                            nc.vector.tensor_tensor(hnew_sb[bt][:, sl], act_cur[3, bt][:],
                                                    t3[:], OP.mult)
                            hb = pb2.tile([128, 512], bf16, tag="hb", name="hb")
                            nc.vector.tensor_copy(hb[:], hnew_sb[bt][:, sl])
                            for j in range(4):
                                ft = t * 4 + j
                                tp = pbtp.tile([128, 128], bf16, tag="tp2", name="tp2")
                                nc.tensor.transpose(tp[:], hb[:, j * 128:(j + 1) * 128],
                                                    identb_sb[:])
                                nc.scalar.copy(hnewT_sb[:, ft, bt * 128:(bt + 1) * 128], tp[:])

            # ============================ phase C ========================
            with tc.tile_pool(name="pcw", bufs=1) as pcw, \
                 tc.tile_pool(name="pctmp", bufs=2) as pctmp:
                qc_sb = pcw.tile([128, NB, BSH], bf16)
                kc_sb = pcw.tile([128, NB, BSH], bf16)
                vc_sb = pcw.tile([128, NB, BSH], bf16)
                exp_sb = pcw.tile([32, NB, BSH], bf16)
                recip_sb = pcw.tile([32, BSH], f32)
                coutb_sb = pcw.tile([128, NB, BSH], bf16)
                wqc_sb = pcw.tile([128, 2, NB, 128], bf16)
                nc.sync.dma_start(out=wqc_sb[:], in_=wqc[:])
                wkc_sb = pcw.tile([128, 2, NB, 128], bf16)
                nc.sync.dma_start(out=wkc_sb[:], in_=wkc[:])
                wvc_sb = pcw.tile([128, 2, NB, 128], bf16)
                nc.sync.dma_start(out=wvc_sb[:], in_=wvc[:])

                with tc.tile_pool(name="pcp1", bufs=2, space="PSUM") as pcp1:
                    for n in range(NB):
                        for wsb, dst in ((wqc_sb, qc_sb), (wkc_sb, kc_sb), (wvc_sb, vc_sb)):
                            ps = pcp1.tile([128, BSH], f32, tag="proj")
                            for s in range(2):
                                nc.tensor.matmul(ps[:], wsb[:, s, n, :],
                                                 hnewT_sb[:, 2 * n + s, :],
                                                 start=(s == 0), stop=(s == 1))
                            nc.scalar.copy(dst[:, n, :], ps[:])

                with tc.tile_pool(name="psS", bufs=1, space="PSUM") as psS:
                    S = psS.tile([32, NB, BSH], f32)
                    for k in range(NB):
                        for q in range(NB):
                            pr = pctmp.tile([128, BSH], bf16, tag="pr")
                            nc.vector.tensor_tensor(pr[:], qc_sb[:, q, :], kc_sb[:, k, :], OP.mult)
                            nc.tensor.matmul(S[:, k, :], hq_sb[:, q, :], pr[:],
                                             start=(q == 0), stop=(q == 7))
                    nc.scalar.activation(exp_sb[:], S[:], AF.Exp,
                                         scale=float(1.0 / np.sqrt(32.0)))
                    denom = pctmp.tile([32, BSH], f32, tag="denom")
                    nc.vector.reduce_sum(denom[:], exp_sb[:].rearrange("p k b -> p b k"),
                                         axis=AX.X)
                    nc.vector.reciprocal(recip_sb[:], denom[:])

                with tc.tile_pool(name="psU", bufs=1, space="PSUM") as psU, \
                     tc.tile_pool(name="psRE", bufs=2, space="PSUM") as psRE:
                    for q in range(NB):
                        U = psU.tile([128, NB, BSH], f32, tag="U")
                        for k in range(NB):
                            nc.tensor.matmul(U[:, k, :], e32b_sb[:, q, :],
                                             exp_sb[:, k, :],
                                             start=True, stop=True)
                        prods = pctmp.tile([128, NB, BSH], f32, tag="prods")
                        nc.vector.tensor_tensor(prods[:], U[:], vc_sb[:], OP.mult)
                        raw = pctmp.tile([128, BSH], f32, tag="raw")
                        nc.vector.reduce_sum(raw[:], prods[:].rearrange("p k b -> p b k"),
                                             axis=AX.X)
                        RE = psRE.tile([128, BSH], f32, tag="RE")
                        nc.tensor.matmul(RE[:], e32f_sb[:, q, :], recip_sb[:],
                                         start=True, stop=True)
                        nc.vector.tensor_tensor(coutb_sb[:, q, :], raw[:], RE[:], OP.mult)

                with tc.tile_pool(name="psOG", bufs=2, space="PSUM") as psOG:
                    for q in range(NB):
                        for bt in range(2):
                            csl = coutb_sb[:, q, bt * 128:(bt + 1) * 128]
                            ops_ = psOG.tile([128, BS], f32, tag="o")
                            nc.tensor.matmul(ops_[:], csl, fcw_sb[:], start=True, stop=False)
                            nc.tensor.matmul(ops_[:], ones1_sb[:], fcb_sb[:], start=False, stop=True)
                            gps_ = psOG.tile([128, BS], f32, tag="gg")
                            nc.tensor.matmul(gps_[:], csl, gw_sb[:], start=True, stop=False)
                            nc.tensor.matmul(gps_[:], ones1_sb[:], gb_sb[:], start=False, stop=True)
                            tano = pctmp.tile([128, BS], f32, tag="tano")
                            nc.scalar.activation(tano[:], ops_[:], AF.Tanh)
                            sg = pctmp.tile([128, BS], f32, tag="sgx")
                            nc.scalar.activation(sg[:], gps_[:], AF.Sigmoid)
                            hatt = pctmp.tile([128, BS], f32, tag="hatt")
                            nc.vector.tensor_tensor(hatt[:], sg[:], tano[:], OP.mult)
                            qsl = slice(q * BS, (q + 1) * BS)
                            nc.vector.tensor_tensor(hnew_sb[bt][:, qsl],
                                                    hnew_sb[bt][:, qsl], hatt[:], OP.add)

            # ============================ phase D ========================
            with tc.tile_pool(name="pd", bufs=2) as pd:
                for bt in range(2):
                    rsl = slice(bt * 128, (bt + 1) * 128)
                    dh = pd.tile([128, NHID], f32, tag="dh")
                    nc.vector.tensor_tensor(dh[:], hnew_sb[bt][:], hx_sb[bt][:], OP.subtract)
                    ho = pd.tile([128, NHID], f32, tag="ho")
                    for n in range(NB):
                        sl = slice(n * BS, (n + 1) * BS)
                        nc.vector.scalar_tensor_tensor(ho[:, sl], dh[:, sl],
                                                       mask_sb[bt][:, n:n + 1],
                                                       hx_sb[bt][:, sl], OP.mult, OP.add)
                    nc.gpsimd.dma_start(out=hx_out[rsl, :], in_=ho[:])

                    dc = pd.tile([128, NHID], f32, tag="dc")
                    nc.vector.tensor_tensor(dc[:], cnew_sb[bt][:], cx_sb[bt][:], OP.subtract)
                    co = pd.tile([128, NHID], f32, tag="co")
                    for n in range(NB):
                        sl = slice(n * BS, (n + 1) * BS)
                        nc.vector.scalar_tensor_tensor(co[:, sl], dc[:, sl],
                                                       mask_sb[bt][:, n:n + 1],
                                                       cx_sb[bt][:, sl], OP.mult, OP.add)
                    nc.gpsimd.dma_start(out=cx_out[rsl, :], in_=co[:])

                    mo = pd.tile([128, NHID], f32, tag="mo")
                    for n in range(NB):
                        sl = slice(n * BS, (n + 1) * BS)
                        nc.vector.tensor_single_scalar(mo[:, sl], ones256_sb[:],
                                                       mask_sb[bt][:, n:n + 1], OP.mult)
                    nc.gpsimd.dma_start(out=mask_out[rsl, :], in_=mo[:])

    _install_bir_fix(nc)
    return nc


# ---------------------------------------------------------------------------
# Host wrapper
# ---------------------------------------------------------------------------

def kernel(inp, hx, cx, wq_inp, wk_inp, wv_inp, w_ih, w_hh, b_ih, b_hh,
           wq_c, wk_c, wv_c, fc_w, fc_b, gate_w, gate_b, step=None):
    global last_exec_time_ns, last_results

    inp = np.asarray(inp, np.float32)
    hx = np.asarray(hx, np.float32)
    cx = np.asarray(cx, np.float32)
    wq_inp = np.asarray(wq_inp, np.float32)
    wk_inp = np.asarray(wk_inp, np.float32)
    wv_inp = np.asarray(wv_inp, np.float32)
    w_ih = np.asarray(w_ih, np.float32)
    w_hh = np.asarray(w_hh, np.float32)
    b_ih = np.asarray(b_ih, np.float32)
    b_hh = np.asarray(b_hh, np.float32)
    wq_c = np.asarray(wq_c, np.float32)
    wk_c = np.asarray(wk_c, np.float32)
    wv_c = np.asarray(wv_c, np.float32)
    fc_w = np.asarray(fc_w, np.float32)
    fc_b = np.asarray(fc_b, np.float32)
    gate_w = np.asarray(gate_w, np.float32)
    gate_b = np.asarray(gate_b, np.float32)

    if "nc" not in _CACHE:
        _CACHE["nc"] = _build()
    nc = _CACHE["nc"]

    # shared (replicated) tensors
    # permute gate columns so 512-wide block j holds gate type j%4 for
    # hidden chunk j//4 (matches the device's incremental LSTM evaluation)
    perm = np.concatenate([np.arange(gt * NHID + t * 512, gt * NHID + (t + 1) * 512)
                           for t in range(4) for gt in range(4)])
    wcat = np.concatenate([w_ih.T, w_hh.T], axis=0)[:, perm]
    wcatT = np.ascontiguousarray(wcat).astype(BF16)
    biasc = (b_ih + b_hh)[perm].astype(BF16).reshape(1, GATES)
    shared = {
        "wq": np.ascontiguousarray(wq_inp.reshape(NB, 2, 128, DKI).transpose(2, 1, 0, 3)),
        "wk1": np.ascontiguousarray(wk_inp[1].reshape(8, 128, DKI).transpose(1, 0, 2)),
        "wv1": np.ascontiguousarray(wv_inp[1].reshape(8, 128, BS).transpose(1, 0, 2)),
        "wcatT": wcatT,
        "biasc": biasc,
        "wqc": np.ascontiguousarray(wq_c.astype(BF16).reshape(NB, 2, 128, 128).transpose(2, 1, 0, 3)),
        "wkc": np.ascontiguousarray(wk_c.astype(BF16).reshape(NB, 2, 128, 128).transpose(2, 1, 0, 3)),
        "wvc": np.ascontiguousarray(wv_c.astype(BF16).reshape(NB, 2, 128, 128).transpose(2, 1, 0, 3)),
        "fcw": fc_w.astype(BF16),
        "gw": gate_w.astype(BF16),
        "fcb": fc_b.astype(BF16).reshape(1, BS),
        "gb": gate_b.astype(BF16).reshape(1, BS),
    }

    in_maps = []
    for c in range(NCORES):
        rs = slice(c * BSH, (c + 1) * BSH)
        inpT = inp[rs].T.reshape(8, 128, BSH).transpose(1, 0, 2)
        hxT = hx[rs].T.reshape(16, 128, BSH).transpose(1, 0, 2)
        m = {
            "inpT": np.ascontiguousarray(inpT),
            "hxT_f": np.ascontiguousarray(hxT),
            "hxT_b": np.ascontiguousarray(hxT.astype(BF16)),
            "hx_bm": np.ascontiguousarray(hx[rs]),
            "cx_bm": np.ascontiguousarray(cx[rs]),
        }
        m.update(shared)
        in_maps.append(m)

    from concourse.bass_utils import run_bass_kernel_spmd
    trace = bool(int(os.environ.get("BASS_KTRACE", "0")))
    res = run_bass_kernel_spmd(nc, in_maps, list(range(NCORES)), trace=trace)
    last_exec_time_ns = res.exec_time_ns
    last_results = res

    hx_full = np.empty((B, NHID), np.float32)
    cx_full = np.empty((B, NHID), np.float32)
    mask_full = np.empty((B, NHID), np.float32)
    for c in range(NCORES):
        rs = slice(c * BSH, (c + 1) * BSH)
        hx_full[rs] = res.results[c]["hx_out"]
        cx_full[rs] = res.results[c]["cx_out"]
        mask_full[rs] = res.results[c]["mask_out"]
    return hx_full, cx_full, mask_full


# revision 12
# speedup vs baseline: 1.0564x; 1.0564x over previous
"""Trainium2 Bass kernel for nn_BlocksCore (RIMs BlocksCore step).

Strategy: data-parallel over batch B=2048 across 8 NeuronCores (256 rows
each). All parameters replicated. Per-core computation:

  1. input attention (f32): k1 = inp@wk1, v1 = inp@wv1, q_n = hx_n@wq_n,
     s1[b,n] = q_n.k1 / 8 (zero-slot score is exactly 0, so softmax over
     [0, s1] collapses to sigmoid);  inp_flat[b, n*256+j] = sig(s1)[b,n]*v1[b,j]
  2. top-k mask: keep the 4 blocks with largest s1 (drop the 4 that attend
     most to the null slot), per row
  3. LSTM cell (bf16 matmuls): gates = [inp_flat|hx] @ [w_ih|w_hh]^T + b
  4. communication attention among the 8 blocks (4 heads, dk=dv=32),
     gated residual: hx_new = h + sigmoid(c@gw+gb)*tanh(c@fw+fb)
  5. masked update of hx/cx.

Layout: batch-major ([batch_p, feat]) for elementwise work; matmuls take
pre-transposed (feature-major) activations as stationary operands. Weights
are pre-transposed/cast on the host. Outputs are batch-major, so the host
just concatenates the 8 shards.
"""

import json
import os

import numpy as np
import ml_dtypes

BF16 = ml_dtypes.bfloat16

B = 2048
NCORES = 8
BSH = B // NCORES          # 256 batch rows per core
NINP = 1024
NHID = 2048
NB = 8                     # blocks
BS = 256                   # block size (NHID / NB)
DKI = 64                   # input-attention d_k
GATES = 4 * NHID           # 8192
KX = 2 * NHID              # LSTM contraction: [inp_flat(2048) | hx(2048)]

_CACHE = {}
last_exec_time_ns = None
last_results = None

# ---------------------------------------------------------------------------
# BIR post-fix: this toolchain's core_v3 codegen supports only one sync-wait
# per CTRL-class instruction (Drain/NoOp/branch). Tile's final drain can carry
# several; hoist extras onto single-wait EventSemaphore instructions.
# ---------------------------------------------------------------------------
# applies to every opcode in this build, so split waits on all of them


def _fix_bir_json(bir_bytes: bytes) -> bytes:
    bir = json.loads(bir_bytes)
    for fn in bir.get("functions", []):
        for blk in fn.get("blocks", []):
            out = []
            for ins in blk.get("instructions", []):
                si = ins.get("sync_info") or {}
                waits = si.get("on_wait") or []
                if len(waits) > 1:
                    for j, w in enumerate(waits[:-1]):
                        out.append({
                            "name": f"{ins['name']}-w{j}",
                            "engine": ins["engine"],
                            "opcode": "EventSemaphore",
                            "ins": [],
                            "outs": [],
                            "sync_info": {"on_update": [], "on_wait": [w]},
                        })
                    si = dict(si)
                    si["on_wait"] = [waits[-1]]
                    ins = dict(ins)
                    ins["sync_info"] = si
                out.append(ins)
            blk["instructions"] = out
    return json.dumps(bir).encode()


def _install_bir_fix(nc):
    orig = nc.to_json_bytes

    def patched(*a, **k):
        return _fix_bir_json(orig(*a, **k))

    nc.to_json_bytes = patched


# ---------------------------------------------------------------------------
# Device kernel
# ---------------------------------------------------------------------------

def _build():
    import concourse.bass as bass
    import concourse.tile as tile
    from concourse import mybir

    f32 = mybir.dt.float32
    bf16 = mybir.dt.bfloat16
    OP = mybir.AluOpType
    AF = mybir.ActivationFunctionType
    AX = mybir.AxisListType

    nc = bass.Bass()

    # ---- I/O ------------------------------------------------------------
    inpT = nc.declare_dram_parameter("inpT", [128, 8, BSH], f32, isOutput=False)
    hxT_f = nc.declare_dram_parameter("hxT_f", [128, 16, BSH], f32, isOutput=False)
    hxT_b = nc.declare_dram_parameter("hxT_b", [128, 16, BSH], bf16, isOutput=False)
    hx_bm = nc.declare_dram_parameter("hx_bm", [BSH, NHID], f32, isOutput=False)
    cx_bm = nc.declare_dram_parameter("cx_bm", [BSH, NHID], f32, isOutput=False)
    wq = nc.declare_dram_parameter("wq", [128, 2, NB, DKI], f32, isOutput=False)
    wk1 = nc.declare_dram_parameter("wk1", [128, 8, DKI], f32, isOutput=False)
    wv1 = nc.declare_dram_parameter("wv1", [128, 8, BS], f32, isOutput=False)
    wcatT = nc.declare_dram_parameter("wcatT", [KX, GATES], bf16, isOutput=False)
    biasc = nc.declare_dram_parameter("biasc", [1, GATES], bf16, isOutput=False)
    wqc = nc.declare_dram_parameter("wqc", [128, 2, NB, 128], bf16, isOutput=False)
    wkc = nc.declare_dram_parameter("wkc", [128, 2, NB, 128], bf16, isOutput=False)
    wvc = nc.declare_dram_parameter("wvc", [128, 2, NB, 128], bf16, isOutput=False)
    fcw = nc.declare_dram_parameter("fcw", [128, BS], bf16, isOutput=False)
    gw = nc.declare_dram_parameter("gw", [128, BS], bf16, isOutput=False)
    fcb = nc.declare_dram_parameter("fcb", [1, BS], bf16, isOutput=False)
    gb = nc.declare_dram_parameter("gb", [1, BS], bf16, isOutput=False)
    hx_out = nc.declare_dram_parameter("hx_out", [BSH, NHID], f32, isOutput=True)
    cx_out = nc.declare_dram_parameter("cx_out", [BSH, NHID], f32, isOutput=True)
    mask_out = nc.declare_dram_parameter("mask_out", [BSH, NHID], f32, isOutput=True)

    # ---- inline constants ----------------------------------------------
    ident_np = np.eye(128, dtype=BF16)
    # score-placement selector: for query block q, out row m = h*8+q gets the
    # head-h sum of a [128]-feature product vector (d -> h = d//32)
    hq_np = np.zeros((128, NB, 32), dtype=BF16)
    for d in range(128):
        for q in range(NB):
            hq_np[d, q, (d // 32) * 8 + q] = 1
    # head expander: for query block q, out feature m (=h*32+d) reads score
    # row r = (m//32)*8 + q
    e32_np = np.zeros((32, NB, 128), dtype=BF16)
    for m in range(128):
        for q in range(NB):
            e32_np[(m // 32) * 8 + q, q, m] = 1
    identb = nc.inline_tensor(ident_np, "identb")
    hqc = nc.inline_tensor(hq_np, "hqc")
    e32b = nc.inline_tensor(e32_np, "e32b")
    e32f = nc.inline_tensor(e32_np.astype(np.float32), "e32f")
    ones1c = nc.inline_tensor(np.ones((1, 128), dtype=BF16), "ones1c")

    with tile.TileContext(nc) as tc:
        with tc.tile_pool(name="cp", bufs=1) as cp, \
             tc.tile_pool(name="pp", bufs=1) as pp:
            # constants to SBUF
            identb_sb = cp.tile([128, 128], bf16)
            nc.sync.dma_start(out=identb_sb[:], in_=identb[:])
            hq_sb = cp.tile([128, NB, 32], bf16)
            nc.sync.dma_start(out=hq_sb[:], in_=hqc[:])
            e32b_sb = cp.tile([32, NB, 128], bf16)
            nc.sync.dma_start(out=e32b_sb[:], in_=e32b[:])
            e32f_sb = cp.tile([32, NB, 128], f32)
            nc.sync.dma_start(out=e32f_sb[:], in_=e32f[:])
            ones1_sb = cp.tile([1, 128], bf16)
            nc.sync.dma_start(out=ones1_sb[:], in_=ones1c[:])
            fcw_sb = cp.tile([128, BS], bf16)
            nc.sync.dma_start(out=fcw_sb[:], in_=fcw[:])
            gw_sb = cp.tile([128, BS], bf16)
            nc.sync.dma_start(out=gw_sb[:], in_=gw[:])
            fcb_sb = cp.tile([1, BS], bf16)
            nc.sync.dma_start(out=fcb_sb[:], in_=fcb[:])
            gb_sb = cp.tile([1, BS], bf16)
            nc.sync.dma_start(out=gb_sb[:], in_=gb[:])
            ones256_sb = cp.tile([128, BS], f32)
            nc.vector.memset(ones256_sb[:], 1.0)

            # persistent inputs / intermediates
            hxTb_sb = pp.tile([128, 16, BSH], bf16)
            nc.sync.dma_start(out=hxTb_sb[:], in_=hxT_b[:])
            hx_sb = [pp.tile([128, NHID], f32, tag=f"hx{bt}", name=f"hx{bt}") for bt in range(2)]
            cx_sb = [pp.tile([128, NHID], f32, tag=f"cx{bt}", name=f"cx{bt}") for bt in range(2)]
            for bt in range(2):
                nc.sync.dma_start(out=hx_sb[bt][:], in_=hx_bm[bt * 128:(bt + 1) * 128, :])
                nc.sync.dma_start(out=cx_sb[bt][:], in_=cx_bm[bt * 128:(bt + 1) * 128, :])

            xt_sb = pp.tile([128, 16, 2, 128], bf16)      # inp_flat^T tiles
            hnew_sb = [pp.tile([128, NHID], f32, tag=f"hn{bt}", name=f"hn{bt}") for bt in range(2)]
            cnew_sb = [pp.tile([128, NHID], f32, tag=f"cn{bt}", name=f"cn{bt}") for bt in range(2)]
            hnewT_sb = pp.tile([128, 16, BSH], bf16)
            mask_sb = [pp.tile([128, NB], f32, tag=f"mk{bt}", name=f"mk{bt}") for bt in range(2)]
            sig_sb = [pp.tile([128, NB], f32, tag=f"sg{bt}", name=f"sg{bt}") for bt in range(2)]

            # ============================ phase A ========================
            with tc.tile_pool(name="pa", bufs=1) as pa, \
                 tc.tile_pool(name="pa2", bufs=2) as pa2, \
                 tc.tile_pool(name="paps", bufs=2, space="PSUM") as paps, \
                 tc.tile_pool(name="patp", bufs=2, space="PSUM") as patp:
                inpT_sb = pa.tile([128, 8, BSH], f32)
                nc.scalar.dma_start(out=inpT_sb[:], in_=inpT[:])
                hxTf_sb = pa.tile([128, 16, BSH], f32)
                nc.scalar.dma_start(out=hxTf_sb[:], in_=hxT_f[:])
                wk1_sb = pa.tile([128, 8, DKI], f32)
                nc.sync.dma_start(out=wk1_sb[:], in_=wk1[:])
                wv1_sb = pa.tile([128, 8, BS], f32)
                nc.sync.dma_start(out=wv1_sb[:], in_=wv1[:])
                wq_sb = pa.tile([128, 2, NB, DKI], f32)
                nc.sync.dma_start(out=wq_sb[:], in_=wq[:])

                for bt in range(2):
                    bsl = slice(bt * 128, (bt + 1) * 128)
                    k1_ps = paps.tile([128, DKI], f32, tag="k1")
                    for k in range(8):
                        nc.tensor.matmul(k1_ps[:], inpT_sb[:, k, bsl], wk1_sb[:, k, :],
                                         start=(k == 0), stop=(k == 7))
                    k1s = pa2.tile([128, DKI], f32, tag="k1s")
                    nc.vector.tensor_copy(k1s[:], k1_ps[:])

                    v1_ps = paps.tile([128, BS], f32, tag="v1")
                    for k in range(8):
                        nc.tensor.matmul(v1_ps[:], inpT_sb[:, k, bsl], wv1_sb[:, k, :],
                                         start=(k == 0), stop=(k == 7))
                    v1s = pa2.tile([128, BS], f32, tag="v1s")
                    nc.vector.tensor_copy(v1s[:], v1_ps[:])

                    q_ps = paps.tile([128, NB, DKI], f32, tag="q")
                    for n in range(NB):
                        for s in range(2):
                            nc.tensor.matmul(q_ps[:, n, :],
                                             hxTf_sb[:, 2 * n + s, bsl],
                                             wq_sb[:, s, n, :],
                                             start=(s == 0), stop=(s == 1))
                    prod = pa2.tile([128, NB, DKI], f32, tag="prod")
                    for n in range(NB):
                        nc.vector.tensor_tensor(prod[:, n, :], q_ps[:, n, :], k1s[:], OP.mult)
                    s1 = pa2.tile([128, NB], f32, tag="s1")
                    nc.vector.reduce_sum(s1[:], prod[:], axis=AX.X)
                    nc.scalar.activation(sig_sb[bt][:], s1[:], AF.Sigmoid, scale=0.125)

                    # top-4 mask: keep blocks whose s1 is among the 4 largest
                    cnt = pa2.tile([128, NB], f32, tag="cnt")
                    tmp = pa2.tile([128, NB], f32, tag="tmp")
                    for n in range(NB):
                        nc.vector.tensor_single_scalar(tmp[:], s1[:], s1[:, n:n + 1], OP.is_gt)
                        nc.vector.reduce_sum(cnt[:, n:n + 1], tmp[:], axis=AX.X)
                    nc.vector.tensor_single_scalar(mask_sb[bt][:], cnt[:], 4.0, OP.is_lt)

                    # inp_flat (batch-major, bf16) then transpose to xt tiles
                    ifl = pa2.tile([128, NB, BS], bf16, tag="ifl")
                    for n in range(NB):
                        nc.vector.tensor_single_scalar(ifl[:, n, :], v1s[:],
                                                       sig_sb[bt][:, n:n + 1], OP.mult)
                    for ft in range(16):
                        tp = patp.tile([128, 128], bf16, tag="tp")
                        nc.tensor.transpose(tp[:], ifl[:, ft // 2, (ft % 2) * 128:(ft % 2) * 128 + 128],
                                            identb_sb[:])
                        nc.scalar.copy(xt_sb[:, ft, bt, :], tp[:])

            # ============================ phase B ========================
            # wcatT/biasc columns are host-permuted: 512-wide block j holds
            # gate type j%4 (0=i,1=f,2=g,3=o) for hidden chunk j//4. Each odd
            # ncp completes a hidden chunk, so gate activations are computed
            # and released incrementally.
            with tc.tile_pool(name="pw", bufs=3) as pw, \
                 tc.tile_pool(name="pact", bufs=2) as pact, \
                 tc.tile_pool(name="pb2", bufs=2) as pb2:
                hnb_sb = [pb2.tile([128, NHID], bf16, tag=f"hnb{bt}", name=f"hnb{bt}",
                                   bufs=1) for bt in range(2)]
                act_cur = {}
                pbps = ctx_b = tc.tile_pool(name="pbps", bufs=2, space="PSUM")
                pbps = pbps.__enter__()
                for ncp in range(8):
                    g = {}
                    for bt in range(2):
                        for h in range(2):
                            g[bt, h] = pbps.tile([128, 512], f32, tag=f"g{bt}{h}", name=f"g{bt}{h}")
                    for k in range(32):
                        w = pw.tile([128, 1024], bf16)
                        eng = nc.sync if (k % 2 == 0) else nc.scalar
                        eng.dma_start(out=w[:],
                                      in_=wcatT[k * 128:(k + 1) * 128,
                                                ncp * 1024:(ncp + 1) * 1024])
                        for bt in range(2):
                            if k < 16:
                                lhsT = xt_sb[:, k, bt, :]
                            else:
                                lhsT = hxTb_sb[:, k - 16, bt * 128:(bt + 1) * 128]
                            for h in range(2):
                                nc.tensor.matmul(g[bt, h][:], lhsT, w[:, h * 512:(h + 1) * 512],
                                                 start=(k == 0), stop=False)
                    bsl_t = pb2.tile([1, 1024], bf16, tag="biasc", name="biascsl")
                    nc.sync.dma_start(out=bsl_t[:], in_=biasc[:, ncp * 1024:(ncp + 1) * 1024])
                    for bt in range(2):
                        for h in range(2):
                            jblk = 2 * ncp + h
                            gt_, t = jblk % 4, jblk // 4
                            nc.tensor.matmul(g[bt, h][:], ones1_sb[:],
                                             bsl_t[:, h * 512:(h + 1) * 512],
                                             start=False, stop=True)
                            a = pact.tile([128, 512], f32, tag=f"a{gt_}{bt}", name=f"a{gt_}{bt}")
                            func = AF.Tanh if gt_ == 2 else AF.Sigmoid
                            nc.scalar.activation(a[:], g[bt, h][:], func)
                            act_cur[gt_, bt] = a
                    if ncp % 2 == 1:
                        t = ncp // 2
                        sl = slice(t * 512, (t + 1) * 512)
                        for bt in range(2):
                            t1 = pb2.tile([128, 512], f32, tag="t1", name="t1")
                            nc.vector.tensor_tensor(t1[:], act_cur[1, bt][:],
                                                    cx_sb[bt][:, sl], OP.mult)
                            t2 = pb2.tile([128, 512], f32, tag="t2", name="t2")
                            nc.vector.tensor_tensor(t2[:], act_cur[0, bt][:],
                                                    act_cur[2, bt][:], OP.mult)
                            nc.vector.tensor_tensor(cnew_sb[bt][:, sl], t1[:], t2[:], OP.add)
                            t3 = pb2.tile([128, 512], f32, tag="t3", name="t3")
                            nc.scalar.activation(t3[:], cnew_sb[bt][:, sl], AF.Tanh)
                            nc.vector.tensor_tensor(hnew_sb[bt][:, sl], act_cur[3, bt][:],
                                                    t3[:], OP.mult)
                            hb = hnb_sb[bt]
                            nc.vector.tensor_copy(hb[:, sl], hnew_sb[bt][:, sl])

                ctx_b.__exit__(None, None, None)
                with tc.tile_pool(name="pbtp", bufs=3, space="PSUM") as pbtp:
                    for bt in range(2):
                        for ft in range(16):
                            tp = pbtp.tile([128, 128], bf16, tag="tp2", name="tp2")
                            nc.tensor.transpose(tp[:], hnb_sb[bt][:, ft * 128:(ft + 1) * 128],
                                                identb_sb[:])
                            nc.scalar.copy(hnewT_sb[:, ft, bt * 128:(bt + 1) * 128], tp[:])

            # ============================ phase C ========================
            with tc.tile_pool(name="pcw", bufs=1) as pcw, \
                 tc.tile_pool(name="pctmp", bufs=2) as pctmp:
                qc_sb = pcw.tile([128, NB, BSH], bf16)
                kc_sb = pcw.tile([128, NB, BSH], bf16)
                vc_sb = pcw.tile([128, NB, BSH], bf16)
                exp_sb = pcw.tile([32, NB, BSH], bf16)
                recip_sb = pcw.tile([32, BSH], f32)
                coutb_sb = pcw.tile([128, NB, BSH], bf16)
                wqc_sb = pcw.tile([128, 2, NB, 128], bf16)
                nc.sync.dma_start(out=wqc_sb[:], in_=wqc[:])
                wkc_sb = pcw.tile([128, 2, NB, 128], bf16)
                nc.sync.dma_start(out=wkc_sb[:], in_=wkc[:])
                wvc_sb = pcw.tile([128, 2, NB, 128], bf16)
                nc.sync.dma_start(out=wvc_sb[:], in_=wvc[:])

                with tc.tile_pool(name="pcp1", bufs=2, space="PSUM") as pcp1:
                    for n in range(NB):
                        for wsb, dst in ((wqc_sb, qc_sb), (wkc_sb, kc_sb), (wvc_sb, vc_sb)):
                            ps = pcp1.tile([128, BSH], f32, tag="proj")
                            for s in range(2):
                                nc.tensor.matmul(ps[:], wsb[:, s, n, :],
                                                 hnewT_sb[:, 2 * n + s, :],
                                                 start=(s == 0), stop=(s == 1))
                            nc.scalar.copy(dst[:, n, :], ps[:])

                with tc.tile_pool(name="psS", bufs=1, space="PSUM") as psS:
                    S = psS.tile([32, NB, BSH], f32)
                    for k in range(NB):
                        for q in range(NB):
                            pr = pctmp.tile([128, BSH], bf16, tag="pr")
                            nc.vector.tensor_tensor(pr[:], qc_sb[:, q, :], kc_sb[:, k, :], OP.mult)
                            nc.tensor.matmul(S[:, k, :], hq_sb[:, q, :], pr[:],
                                             start=(q == 0), stop=(q == 7))
                    nc.scalar.activation(exp_sb[:], S[:], AF.Exp,
                                         scale=float(1.0 / np.sqrt(32.0)))
                    denom = pctmp.tile([32, BSH], f32, tag="denom")
                    nc.vector.reduce_sum(denom[:], exp_sb[:].rearrange("p k b -> p b k"),
                                         axis=AX.X)
                    nc.vector.reciprocal(recip_sb[:], denom[:])

                with tc.tile_pool(name="psU", bufs=1, space="PSUM") as psU, \
                     tc.tile_pool(name="psRE", bufs=2, space="PSUM") as psRE:
                    for q in range(NB):
                        U = psU.tile([128, NB, BSH], f32, tag="U")
                        for k in range(NB):
                            nc.tensor.matmul(U[:, k, :], e32b_sb[:, q, :],
                                             exp_sb[:, k, :],
                                             start=True, stop=True)
                        prods = pctmp.tile([128, NB, BSH], f32, tag="prods")
                        nc.vector.tensor_tensor(prods[:], U[:], vc_sb[:], OP.mult)
                        raw = pctmp.tile([128, BSH], f32, tag="raw")
                        nc.vector.reduce_sum(raw[:], prods[:].rearrange("p k b -> p b k"),
                                             axis=AX.X)
                        RE = psRE.tile([128, BSH], f32, tag="RE")
                        nc.tensor.matmul(RE[:], e32f_sb[:, q, :], recip_sb[:],
                                         start=True, stop=True)
                        nc.vector.tensor_tensor(coutb_sb[:, q, :], raw[:], RE[:], OP.mult)

                with tc.tile_pool(name="psOG", bufs=2, space="PSUM") as psOG:
                    for q in range(NB):
                        for bt in range(2):
                            csl = coutb_sb[:, q, bt * 128:(bt + 1) * 128]
                            ops_ = psOG.tile([128, BS], f32, tag="o")
                            nc.tensor.matmul(ops_[:], csl, fcw_sb[:], start=True, stop=False)
                            nc.tensor.matmul(ops_[:], ones1_sb[:], fcb_sb[:], start=False, stop=True)
                            gps_ = psOG.tile([128, BS], f32, tag="gg")
                            nc.tensor.matmul(gps_[:], csl, gw_sb[:], start=True, stop=False)
                            nc.tensor.matmul(gps_[:], ones1_sb[:], gb_sb[:], start=False, stop=True)
                            tano = pctmp.tile([128, BS], f32, tag="tano")
                            nc.scalar.activation(tano[:], ops_[:], AF.Tanh)
                            sg = pctmp.tile([128, BS], f32, tag="sgx")
                            nc.scalar.activation(sg[:], gps_[:], AF.Sigmoid)
                            hatt = pctmp.tile([128, BS], f32, tag="hatt")
                            nc.vector.tensor_tensor(hatt[:], sg[:], tano[:], OP.mult)
                            qsl = slice(q * BS, (q + 1) * BS)
                            nc.vector.tensor_tensor(hnew_sb[bt][:, qsl],
                                                    hnew_sb[bt][:, qsl], hatt[:], OP.add)

            # ============================ phase D ========================
            with tc.tile_pool(name="pd", bufs=2) as pd:
                for bt in range(2):
                    rsl = slice(bt * 128, (bt + 1) * 128)
                    dh = pd.tile([128, NHID], f32, tag="dh")
                    nc.vector.tensor_tensor(dh[:], hnew_sb[bt][:], hx_sb[bt][:], OP.subtract)
                    ho = pd.tile([128, NHID], f32, tag="ho")
                    for n in range(NB):
                        sl = slice(n * BS, (n + 1) * BS)
                        nc.vector.scalar_tensor_tensor(ho[:, sl], dh[:, sl],
                                                       mask_sb[bt][:, n:n + 1],
                                                       hx_sb[bt][:, sl], OP.mult, OP.add)
                    nc.sync.dma_start(out=hx_out[rsl, :], in_=ho[:])

                    dc = pd.tile([128, NHID], f32, tag="dc")
                    nc.vector.tensor_tensor(dc[:], cnew_sb[bt][:], cx_sb[bt][:], OP.subtract)
                    co = pd.tile([128, NHID], f32, tag="co")
                    for n in range(NB):
                        sl = slice(n * BS, (n + 1) * BS)
                        nc.vector.scalar_tensor_tensor(co[:, sl], dc[:, sl],
                                                       mask_sb[bt][:, n:n + 1],
                                                       cx_sb[bt][:, sl], OP.mult, OP.add)
                    nc.sync.dma_start(out=cx_out[rsl, :], in_=co[:])

                    mo = pd.tile([128, NHID], f32, tag="mo")
                    for n in range(NB):
                        sl = slice(n * BS, (n + 1) * BS)
                        nc.vector.tensor_single_scalar(mo[:, sl], ones256_sb[:],
                                                       mask_sb[bt][:, n:n + 1], OP.mult)
                    nc.sync.dma_start(out=mask_out[rsl, :], in_=mo[:])

    _install_bir_fix(nc)
    return nc


# ---------------------------------------------------------------------------
# Host wrapper
# ---------------------------------------------------------------------------

def kernel(inp, hx, cx, wq_inp, wk_inp, wv_inp, w_ih, w_hh, b_ih, b_hh,
           wq_c, wk_c, wv_c, fc_w, fc_b, gate_w, gate_b, step=None):
    global last_exec_time_ns, last_results

    inp = np.asarray(inp, np.float32)
    hx = np.asarray(hx, np.float32)
    cx = np.asarray(cx, np.float32)
    wq_inp = np.asarray(wq_inp, np.float32)
    wk_inp = np.asarray(wk_inp, np.float32)
    wv_inp = np.asarray(wv_inp, np.float32)
    w_ih = np.asarray(w_ih, np.float32)
    w_hh = np.asarray(w_hh, np.float32)
    b_ih = np.asarray(b_ih, np.float32)
    b_hh = np.asarray(b_hh, np.float32)
    wq_c = np.asarray(wq_c, np.float32)
    wk_c = np.asarray(wk_c, np.float32)
    wv_c = np.asarray(wv_c, np.float32)
    fc_w = np.asarray(fc_w, np.float32)
    fc_b = np.asarray(fc_b, np.float32)
    gate_w = np.asarray(gate_w, np.float32)
    gate_b = np.asarray(gate_b, np.float32)

    if "nc" not in _CACHE:
        _CACHE["nc"] = _build()
    nc = _CACHE["nc"]

    # shared (replicated) tensors
    # permute gate columns so 512-wide block j holds gate type j%4 for
    # hidden chunk j//4 (matches the device's incremental LSTM evaluation)
    perm = np.concatenate([np.arange(gt * NHID + t * 512, gt * NHID + (t + 1) * 512)
                           for t in range(4) for gt in range(4)])
    wcat = np.concatenate([w_ih.T, w_hh.T], axis=0)[:, perm]
    wcatT = np.ascontiguousarray(wcat).astype(BF16)
    biasc = (b_ih + b_hh)[perm].astype(BF16).reshape(1, GATES)
    shared = {
        "wq": np.ascontiguousarray(wq_inp.reshape(NB, 2, 128, DKI).transpose(2, 1, 0, 3)),
        "wk1": np.ascontiguousarray(wk_inp[1].reshape(8, 128, DKI).transpose(1, 0, 2)),
        "wv1": np.ascontiguousarray(wv_inp[1].reshape(8, 128, BS).transpose(1, 0, 2)),
        "wcatT": wcatT,
        "biasc": biasc,
        "wqc": np.ascontiguousarray(wq_c.astype(BF16).reshape(NB, 2, 128, 128).transpose(2, 1, 0, 3)),
        "wkc": np.ascontiguousarray(wk_c.astype(BF16).reshape(NB, 2, 128, 128).transpose(2, 1, 0, 3)),
        "wvc": np.ascontiguousarray(wv_c.astype(BF16).reshape(NB, 2, 128, 128).transpose(2, 1, 0, 3)),
        "fcw": fc_w.astype(BF16),
        "gw": gate_w.astype(BF16),
        "fcb": fc_b.astype(BF16).reshape(1, BS),
        "gb": gate_b.astype(BF16).reshape(1, BS),
    }

    in_maps = []
    for c in range(NCORES):
        rs = slice(c * BSH, (c + 1) * BSH)
        inpT = inp[rs].T.reshape(8, 128, BSH).transpose(1, 0, 2)
        hxT = hx[rs].T.reshape(16, 128, BSH).transpose(1, 0, 2)
        m = {
            "inpT": np.ascontiguousarray(inpT),
            "hxT_f": np.ascontiguousarray(hxT),
            "hxT_b": np.ascontiguousarray(hxT.astype(BF16)),
            "hx_bm": np.ascontiguousarray(hx[rs]),
            "cx_bm": np.ascontiguousarray(cx[rs]),
        }
        m.update(shared)
        in_maps.append(m)

    from concourse.bass_utils import run_bass_kernel_spmd
    trace = bool(int(os.environ.get("BASS_KTRACE", "0")))
    res = run_bass_kernel_spmd(nc, in_maps, list(range(NCORES)), trace=trace)
    last_exec_time_ns = res.exec_time_ns
    last_results = res

    hx_full = np.empty((B, NHID), np.float32)
    cx_full = np.empty((B, NHID), np.float32)
    mask_full = np.empty((B, NHID), np.float32)
    for c in range(NCORES):
        rs = slice(c * BSH, (c + 1) * BSH)
        hx_full[rs] = res.results[c]["hx_out"]
        cx_full[rs] = res.results[c]["cx_out"]
        mask_full[rs] = res.results[c]["mask_out"]
    return hx_full, cx_full, mask_full


# revision 13
# speedup vs baseline: 1.1341x; 1.0735x over previous
"""Trainium2 Bass kernel for nn_BlocksCore (RIMs BlocksCore step).

Strategy: data-parallel over batch B=2048 across 8 NeuronCores (256 rows
each). All parameters replicated. Per-core computation:

  1. input attention (f32): k1 = inp@wk1, v1 = inp@wv1, q_n = hx_n@wq_n,
     s1[b,n] = q_n.k1 / 8 (zero-slot score is exactly 0, so softmax over
     [0, s1] collapses to sigmoid);  inp_flat[b, n*256+j] = sig(s1)[b,n]*v1[b,j]
  2. top-k mask: keep the 4 blocks with largest s1 (drop the 4 that attend
     most to the null slot), per row
  3. LSTM cell (bf16 matmuls): gates = [inp_flat|hx] @ [w_ih|w_hh]^T + b
  4. communication attention among the 8 blocks (4 heads, dk=dv=32),
     gated residual: hx_new = h + sigmoid(c@gw+gb)*tanh(c@fw+fb)
  5. masked update of hx/cx.

Layout: batch-major ([batch_p, feat]) for elementwise work; matmuls take
pre-transposed (feature-major) activations as stationary operands. Weights
are pre-transposed/cast on the host. Outputs are batch-major, so the host
just concatenates the 8 shards.
"""

import json
import os

import numpy as np
import ml_dtypes

BF16 = ml_dtypes.bfloat16

B = 2048
NCORES = 8
BSH = B // NCORES          # 256 batch rows per core
NINP = 1024
NHID = 2048
NB = 8                     # blocks
BS = 256                   # block size (NHID / NB)
DKI = 64                   # input-attention d_k
GATES = 4 * NHID           # 8192
KX = 2 * NHID              # LSTM contraction: [inp_flat(2048) | hx(2048)]

_CACHE = {}
last_exec_time_ns = None
last_results = None

# ---------------------------------------------------------------------------
# BIR post-fix: this toolchain's core_v3 codegen supports only one sync-wait
# per CTRL-class instruction (Drain/NoOp/branch). Tile's final drain can carry
# several; hoist extras onto single-wait EventSemaphore instructions.
# ---------------------------------------------------------------------------
# applies to every opcode in this build, so split waits on all of them


def _fix_bir_json(bir_bytes: bytes) -> bytes:
    bir = json.loads(bir_bytes)
    for fn in bir.get("functions", []):
        for blk in fn.get("blocks", []):
            out = []
            for ins in blk.get("instructions", []):
                si = ins.get("sync_info") or {}
                waits = si.get("on_wait") or []
                if len(waits) > 1:
                    for j, w in enumerate(waits[:-1]):
                        out.append({
                            "name": f"{ins['name']}-w{j}",
                            "engine": ins["engine"],
                            "opcode": "EventSemaphore",
                            "ins": [],
                            "outs": [],
                            "sync_info": {"on_update": [], "on_wait": [w]},
                        })
                    si = dict(si)
                    si["on_wait"] = [waits[-1]]
                    ins = dict(ins)
                    ins["sync_info"] = si
                out.append(ins)
            blk["instructions"] = out
    return json.dumps(bir).encode()


def _install_bir_fix(nc):
    orig = nc.to_json_bytes

    def patched(*a, **k):
        return _fix_bir_json(orig(*a, **k))

    nc.to_json_bytes = patched


# ---------------------------------------------------------------------------
# Device kernel
# ---------------------------------------------------------------------------

def _build():
    import concourse.bass as bass
    import concourse.tile as tile
    from concourse import mybir

    f32 = mybir.dt.float32
    bf16 = mybir.dt.bfloat16
    OP = mybir.AluOpType
    AF = mybir.ActivationFunctionType
    AX = mybir.AxisListType

    nc = bass.Bass()

    # ---- I/O ------------------------------------------------------------
    inpT = nc.declare_dram_parameter("inpT", [128, 8, BSH], f32, isOutput=False)
    hxT_f = nc.declare_dram_parameter("hxT_f", [128, 16, BSH], f32, isOutput=False)
    hxT_b = nc.declare_dram_parameter("hxT_b", [128, 16, BSH], bf16, isOutput=False)
    hx_bm = nc.declare_dram_parameter("hx_bm", [BSH, NHID], f32, isOutput=False)
    cx_bm = nc.declare_dram_parameter("cx_bm", [BSH, NHID], f32, isOutput=False)
    wq = nc.declare_dram_parameter("wq", [128, 2, NB, DKI], f32, isOutput=False)
    wk1 = nc.declare_dram_parameter("wk1", [128, 8, DKI], f32, isOutput=False)
    wv1 = nc.declare_dram_parameter("wv1", [128, 8, BS], f32, isOutput=False)
    wcatT = nc.declare_dram_parameter("wcatT", [KX, GATES], bf16, isOutput=False)
    biasc = nc.declare_dram_parameter("biasc", [1, GATES], bf16, isOutput=False)
    wqc = nc.declare_dram_parameter("wqc", [128, 2, NB, 128], bf16, isOutput=False)
    wkc = nc.declare_dram_parameter("wkc", [128, 2, NB, 128], bf16, isOutput=False)
    wvc = nc.declare_dram_parameter("wvc", [128, 2, NB, 128], bf16, isOutput=False)
    fcw = nc.declare_dram_parameter("fcw", [128, BS], bf16, isOutput=False)
    gw = nc.declare_dram_parameter("gw", [128, BS], bf16, isOutput=False)
    fcb = nc.declare_dram_parameter("fcb", [1, BS], bf16, isOutput=False)
    gb = nc.declare_dram_parameter("gb", [1, BS], bf16, isOutput=False)
    hx_out = nc.declare_dram_parameter("hx_out", [BSH, NHID], f32, isOutput=True)
    cx_out = nc.declare_dram_parameter("cx_out", [BSH, NHID], f32, isOutput=True)
    mask_out = nc.declare_dram_parameter("mask_out", [BSH, NHID], f32, isOutput=True)

    # ---- inline constants ----------------------------------------------
    ident_np = np.eye(128, dtype=BF16)
    # score-placement selector: for query block q, out row m = h*8+q gets the
    # head-h sum of a [128]-feature product vector (d -> h = d//32)
    hq_np = np.zeros((128, NB, 32), dtype=BF16)
    for d in range(128):
        for q in range(NB):
            hq_np[d, q, (d // 32) * 8 + q] = 1
    # head expander: for query block q, out feature m (=h*32+d) reads score
    # row r = (m//32)*8 + q
    e32_np = np.zeros((32, NB, 128), dtype=BF16)
    for m in range(128):
        for q in range(NB):
            e32_np[(m // 32) * 8 + q, q, m] = 1
    identb = nc.inline_tensor(ident_np, "identb")
    hqc = nc.inline_tensor(hq_np, "hqc")
    e32b = nc.inline_tensor(e32_np, "e32b")
    e32f = nc.inline_tensor(e32_np.astype(np.float32), "e32f")
    ones1c = nc.inline_tensor(np.ones((1, 128), dtype=BF16), "ones1c")

    with tile.TileContext(nc) as tc:
        with tc.tile_pool(name="cp", bufs=1) as cp, \
             tc.tile_pool(name="pp", bufs=1) as pp:
            # constants to SBUF
            identb_sb = cp.tile([128, 128], bf16)
            nc.sync.dma_start(out=identb_sb[:], in_=identb[:])
            hq_sb = cp.tile([128, NB, 32], bf16)
            nc.sync.dma_start(out=hq_sb[:], in_=hqc[:])
            e32b_sb = cp.tile([32, NB, 128], bf16)
            nc.sync.dma_start(out=e32b_sb[:], in_=e32b[:])
            e32f_sb = cp.tile([32, NB, 128], f32)
            nc.sync.dma_start(out=e32f_sb[:], in_=e32f[:])
            ones1_sb = cp.tile([1, 128], bf16)
            nc.sync.dma_start(out=ones1_sb[:], in_=ones1c[:])
            fcw_sb = cp.tile([128, BS], bf16)
            nc.sync.dma_start(out=fcw_sb[:], in_=fcw[:])
            gw_sb = cp.tile([128, BS], bf16)
            nc.sync.dma_start(out=gw_sb[:], in_=gw[:])
            fcb_sb = cp.tile([1, BS], bf16)
            nc.sync.dma_start(out=fcb_sb[:], in_=fcb[:])
            gb_sb = cp.tile([1, BS], bf16)
            nc.sync.dma_start(out=gb_sb[:], in_=gb[:])
            ones256_sb = cp.tile([128, BS], f32)
            nc.vector.memset(ones256_sb[:], 1.0)

            # persistent inputs / intermediates
            hxTb_sb = pp.tile([128, 16, BSH], bf16)
            nc.sync.dma_start(out=hxTb_sb[:], in_=hxT_b[:])
            hx_sb = [pp.tile([128, NHID], f32, tag=f"hx{bt}", name=f"hx{bt}") for bt in range(2)]
            cx_sb = [pp.tile([128, NHID], f32, tag=f"cx{bt}", name=f"cx{bt}") for bt in range(2)]
            for bt in range(2):
                nc.sync.dma_start(out=hx_sb[bt][:], in_=hx_bm[bt * 128:(bt + 1) * 128, :])
                nc.sync.dma_start(out=cx_sb[bt][:], in_=cx_bm[bt * 128:(bt + 1) * 128, :])

            xt_sb = pp.tile([128, 16, 2, 128], bf16)      # inp_flat^T tiles
            hnew_sb = [pp.tile([128, NHID], f32, tag=f"hn{bt}", name=f"hn{bt}") for bt in range(2)]
            cnew_sb = [pp.tile([128, NHID], f32, tag=f"cn{bt}", name=f"cn{bt}") for bt in range(2)]
            hnewT_sb = pp.tile([128, 16, BSH], bf16)
            mask_sb = [pp.tile([128, NB], f32, tag=f"mk{bt}", name=f"mk{bt}") for bt in range(2)]
            sig_sb = [pp.tile([128, NB], f32, tag=f"sg{bt}", name=f"sg{bt}") for bt in range(2)]

            # ============================ phase A ========================
            with tc.tile_pool(name="pa", bufs=1) as pa, \
                 tc.tile_pool(name="pa2", bufs=2) as pa2, \
                 tc.tile_pool(name="paps", bufs=2, space="PSUM") as paps, \
                 tc.tile_pool(name="patp", bufs=2, space="PSUM") as patp:
                inpT_sb = pa.tile([128, 8, BSH], f32)
                nc.scalar.dma_start(out=inpT_sb[:], in_=inpT[:])
                hxTf_sb = pa.tile([128, 16, BSH], f32)
                nc.scalar.dma_start(out=hxTf_sb[:], in_=hxT_f[:])
                wk1_sb = pa.tile([128, 8, DKI], f32)
                nc.sync.dma_start(out=wk1_sb[:], in_=wk1[:])
                wv1_sb = pa.tile([128, 8, BS], f32)
                nc.sync.dma_start(out=wv1_sb[:], in_=wv1[:])
                wq_sb = pa.tile([128, 2, NB, DKI], f32)
                nc.sync.dma_start(out=wq_sb[:], in_=wq[:])

                for bt in range(2):
                    bsl = slice(bt * 128, (bt + 1) * 128)
                    k1_ps = paps.tile([128, DKI], f32, tag="k1")
                    for k in range(8):
                        nc.tensor.matmul(k1_ps[:], inpT_sb[:, k, bsl], wk1_sb[:, k, :],
                                         start=(k == 0), stop=(k == 7))
                    k1s = pa2.tile([128, DKI], f32, tag="k1s")
                    nc.vector.tensor_copy(k1s[:], k1_ps[:])

                    v1_ps = paps.tile([128, BS], f32, tag="v1")
                    for k in range(8):
                        nc.tensor.matmul(v1_ps[:], inpT_sb[:, k, bsl], wv1_sb[:, k, :],
                                         start=(k == 0), stop=(k == 7))
                    v1s = pa2.tile([128, BS], f32, tag="v1s")
                    nc.vector.tensor_copy(v1s[:], v1_ps[:])

                    q_ps = paps.tile([128, NB, DKI], f32, tag="q")
                    for n in range(NB):
                        for s in range(2):
                            nc.tensor.matmul(q_ps[:, n, :],
                                             hxTf_sb[:, 2 * n + s, bsl],
                                             wq_sb[:, s, n, :],
                                             start=(s == 0), stop=(s == 1))
                    prod = pa2.tile([128, NB, DKI], f32, tag="prod")
                    for n in range(NB):
                        nc.vector.tensor_tensor(prod[:, n, :], q_ps[:, n, :], k1s[:], OP.mult)
                    s1 = pa2.tile([128, NB], f32, tag="s1")
                    nc.vector.reduce_sum(s1[:], prod[:], axis=AX.X)
                    nc.scalar.activation(sig_sb[bt][:], s1[:], AF.Sigmoid, scale=0.125)

                    # top-4 mask: keep blocks whose s1 is among the 4 largest
                    cnt = pa2.tile([128, NB], f32, tag="cnt")
                    tmp = pa2.tile([128, NB], f32, tag="tmp")
                    for n in range(NB):
                        nc.vector.tensor_single_scalar(tmp[:], s1[:], s1[:, n:n + 1], OP.is_gt)
                        nc.vector.reduce_sum(cnt[:, n:n + 1], tmp[:], axis=AX.X)
                    nc.vector.tensor_single_scalar(mask_sb[bt][:], cnt[:], 4.0, OP.is_lt)

                    # inp_flat (batch-major, bf16) then transpose to xt tiles
                    ifl = pa2.tile([128, NB, BS], bf16, tag="ifl")
                    for n in range(NB):
                        nc.vector.tensor_single_scalar(ifl[:, n, :], v1s[:],
                                                       sig_sb[bt][:, n:n + 1], OP.mult)
                    for ft in range(16):
                        tp = patp.tile([128, 128], bf16, tag="tp")
                        nc.tensor.transpose(tp[:], ifl[:, ft // 2, (ft % 2) * 128:(ft % 2) * 128 + 128],
                                            identb_sb[:])
                        nc.scalar.copy(xt_sb[:, ft, bt, :], tp[:])

            # ============================ phase B ========================
            # wcatT/biasc columns are host-permuted: 512-wide block j holds
            # gate type j%4 (0=i,1=f,2=g,3=o) for hidden chunk j//4. One
            # 2048-col group = all four gates of one hidden chunk, so the
            # LSTM elementwise runs incrementally per group. The hx half of
            # the contraction (k>=16) runs first so PE can start before the
            # input-attention phase finishes producing inp_flat^T.
            with tc.tile_pool(name="pw", bufs=3) as pw, \
                 tc.tile_pool(name="pact", bufs=2) as pact, \
                 tc.tile_pool(name="pb2", bufs=2) as pb2:
                hnb_sb = [pb2.tile([128, NHID], bf16, tag=f"hnb{bt}", name=f"hnb{bt}",
                                   bufs=1) for bt in range(2)]
                ks_order = list(range(16, 32)) + list(range(16))
                pbps_cm = tc.tile_pool(name="pbps", bufs=1, space="PSUM")
                pbps = pbps_cm.__enter__()
                for ncpp in range(4):
                    g = {}
                    for bt in range(2):
                        for c in range(4):
                            g[bt, c] = pbps.tile([128, 512], f32, tag=f"g{bt}{c}",
                                                 name=f"g{bt}{c}")
                    for ki, k in enumerate(ks_order):
                        w = pw.tile([128, 2048], bf16)
                        eng = nc.sync if (ki % 2 == 0) else nc.scalar
                        eng.dma_start(out=w[:],
                                      in_=wcatT[k * 128:(k + 1) * 128,
                                                ncpp * 2048:(ncpp + 1) * 2048])
                        for bt in range(2):
                            if k < 16:
                                lhsT = xt_sb[:, k, bt, :]
                            else:
                                lhsT = hxTb_sb[:, k - 16, bt * 128:(bt + 1) * 128]
                            for c in range(4):
                                nc.tensor.matmul(g[bt, c][:], lhsT,
                                                 w[:, c * 512:(c + 1) * 512],
                                                 start=(ki == 0), stop=False)
                    bsl_t = pb2.tile([1, 2048], bf16, tag="biasc", name="biascsl")
                    nc.sync.dma_start(out=bsl_t[:],
                                      in_=biasc[:, ncpp * 2048:(ncpp + 1) * 2048])
                    act_cur = {}
                    for bt in range(2):
                        for c in range(4):
                            nc.tensor.matmul(g[bt, c][:], ones1_sb[:],
                                             bsl_t[:, c * 512:(c + 1) * 512],
                                             start=False, stop=True)
                            a = pact.tile([128, 512], f32, tag=f"a{c}{bt}", name=f"a{c}{bt}")
                            func = AF.Tanh if c == 2 else AF.Sigmoid
                            nc.scalar.activation(a[:], g[bt, c][:], func)
                            act_cur[c, bt] = a
                    t = ncpp
                    sl = slice(t * 512, (t + 1) * 512)
                    for bt in range(2):
                        t1 = pb2.tile([128, 512], f32, tag="t1", name="t1")
                        nc.vector.tensor_tensor(t1[:], act_cur[1, bt][:],
                                                cx_sb[bt][:, sl], OP.mult)
                        t2 = pb2.tile([128, 512], f32, tag="t2", name="t2")
                        nc.vector.tensor_tensor(t2[:], act_cur[0, bt][:],
                                                act_cur[2, bt][:], OP.mult)
                        nc.vector.tensor_tensor(cnew_sb[bt][:, sl], t1[:], t2[:], OP.add)
                        t3 = pb2.tile([128, 512], f32, tag="t3", name="t3")
                        nc.scalar.activation(t3[:], cnew_sb[bt][:, sl], AF.Tanh)
                        nc.vector.tensor_tensor(hnew_sb[bt][:, sl], act_cur[3, bt][:],
                                                t3[:], OP.mult)
                        nc.vector.tensor_copy(hnb_sb[bt][:, sl], hnew_sb[bt][:, sl])
                pbps_cm.__exit__(None, None, None)

                # cx/mask outputs no longer depend on anything downstream:
                # emit them now so they overlap the communication attention.
                for bt in range(2):
                    rsl = slice(bt * 128, (bt + 1) * 128)
                    dc = pb2.tile([128, NHID], f32, tag="dc", name="dc")
                    nc.vector.tensor_tensor(dc[:], cnew_sb[bt][:], cx_sb[bt][:], OP.subtract)
                    co = pb2.tile([128, NHID], f32, tag="co", name="co", bufs=1)
                    mo = pb2.tile([128, NHID], f32, tag="mo", name="mo", bufs=1)
                    for n in range(NB):
                        sl = slice(n * BS, (n + 1) * BS)
                        nc.vector.scalar_tensor_tensor(co[:, sl], dc[:, sl],
                                                       mask_sb[bt][:, n:n + 1],
                                                       cx_sb[bt][:, sl], OP.mult, OP.add)
                        nc.gpsimd.tensor_single_scalar(mo[:, sl], ones256_sb[:],
                                                       mask_sb[bt][:, n:n + 1], OP.mult)
                    nc.sync.dma_start(out=cx_out[rsl, :], in_=co[:])
                    nc.sync.dma_start(out=mask_out[rsl, :], in_=mo[:])

                with tc.tile_pool(name="pbtp", bufs=3, space="PSUM") as pbtp:
                    for bt in range(2):
                        for ft in range(16):
                            tp = pbtp.tile([128, 128], bf16, tag="tp2", name="tp2")
                            nc.tensor.transpose(tp[:], hnb_sb[bt][:, ft * 128:(ft + 1) * 128],
                                                identb_sb[:])
                            nc.scalar.copy(hnewT_sb[:, ft, bt * 128:(bt + 1) * 128], tp[:])

            # ============================ phase C ========================
            with tc.tile_pool(name="pcw", bufs=1) as pcw, \
                 tc.tile_pool(name="pctmp", bufs=2) as pctmp:
                qc_sb = pcw.tile([128, NB, BSH], bf16)
                kc_sb = pcw.tile([128, NB, BSH], bf16)
                vc_sb = pcw.tile([128, NB, BSH], bf16)
                exp_sb = pcw.tile([32, NB, BSH], bf16)
                recip_sb = pcw.tile([32, BSH], f32)
                coutb_sb = pcw.tile([128, NB, BSH], bf16)
                wqc_sb = pcw.tile([128, 2, NB, 128], bf16)
                nc.sync.dma_start(out=wqc_sb[:], in_=wqc[:])
                wkc_sb = pcw.tile([128, 2, NB, 128], bf16)
                nc.sync.dma_start(out=wkc_sb[:], in_=wkc[:])
                wvc_sb = pcw.tile([128, 2, NB, 128], bf16)
                nc.sync.dma_start(out=wvc_sb[:], in_=wvc[:])

                with tc.tile_pool(name="pcp1", bufs=2, space="PSUM") as pcp1:
                    for n in range(NB):
                        for wsb, dst in ((wqc_sb, qc_sb), (wkc_sb, kc_sb), (wvc_sb, vc_sb)):
                            ps = pcp1.tile([128, BSH], f32, tag="proj")
                            for s in range(2):
                                nc.tensor.matmul(ps[:], wsb[:, s, n, :],
                                                 hnewT_sb[:, 2 * n + s, :],
                                                 start=(s == 0), stop=(s == 1))
                            nc.scalar.copy(dst[:, n, :], ps[:])

                with tc.tile_pool(name="psS", bufs=1, space="PSUM") as psS:
                    S = psS.tile([32, NB, BSH], f32)
                    for k in range(NB):
                        for q in range(NB):
                            pr = pctmp.tile([128, BSH], bf16, tag="pr")
                            nc.vector.tensor_tensor(pr[:], qc_sb[:, q, :], kc_sb[:, k, :], OP.mult)
                            nc.tensor.matmul(S[:, k, :], hq_sb[:, q, :], pr[:],
                                             start=(q == 0), stop=(q == 7))
                    nc.scalar.activation(exp_sb[:], S[:], AF.Exp,
                                         scale=float(1.0 / np.sqrt(32.0)))
                    denom = pctmp.tile([32, BSH], f32, tag="denom")
                    nc.vector.reduce_sum(denom[:], exp_sb[:].rearrange("p k b -> p b k"),
                                         axis=AX.X)
                    nc.vector.reciprocal(recip_sb[:], denom[:])

                with tc.tile_pool(name="psU", bufs=1, space="PSUM") as psU, \
                     tc.tile_pool(name="psRE", bufs=2, space="PSUM") as psRE:
                    for q in range(NB):
                        U = psU.tile([128, NB, BSH], f32, tag="U")
                        for kp in range(4):
                            nc.tensor.matmul(U[:, 2 * kp:2 * kp + 2, :], e32b_sb[:, q, :],
                                             exp_sb[:, 2 * kp:2 * kp + 2, :],
                                             start=True, stop=True)
                        prods = pctmp.tile([128, NB, BSH], f32, tag="prods")
                        nc.vector.tensor_tensor(prods[:], U[:], vc_sb[:], OP.mult)
                        raw = pctmp.tile([128, BSH], f32, tag="raw")
                        nc.vector.reduce_sum(raw[:], prods[:].rearrange("p k b -> p b k"),
                                             axis=AX.X)
                        RE = psRE.tile([128, BSH], f32, tag="RE")
                        nc.tensor.matmul(RE[:], e32f_sb[:, q, :], recip_sb[:],
                                         start=True, stop=True)
                        nc.vector.tensor_tensor(coutb_sb[:, q, :], raw[:], RE[:], OP.mult)

                with tc.tile_pool(name="psOG", bufs=2, space="PSUM") as psOG:
                    for q in range(NB):
                        for bt in range(2):
                            csl = coutb_sb[:, q, bt * 128:(bt + 1) * 128]
                            ops_ = psOG.tile([128, BS], f32, tag="o")
                            nc.tensor.matmul(ops_[:], csl, fcw_sb[:], start=True, stop=False)
                            nc.tensor.matmul(ops_[:], ones1_sb[:], fcb_sb[:], start=False, stop=True)
                            gps_ = psOG.tile([128, BS], f32, tag="gg")
                            nc.tensor.matmul(gps_[:], csl, gw_sb[:], start=True, stop=False)
                            nc.tensor.matmul(gps_[:], ones1_sb[:], gb_sb[:], start=False, stop=True)
                            tano = pctmp.tile([128, BS], f32, tag="tano")
                            nc.scalar.activation(tano[:], ops_[:], AF.Tanh)
                            sg = pctmp.tile([128, BS], f32, tag="sgx")
                            nc.scalar.activation(sg[:], gps_[:], AF.Sigmoid)
                            hatt = pctmp.tile([128, BS], f32, tag="hatt")
                            nc.vector.tensor_tensor(hatt[:], sg[:], tano[:], OP.mult)
                            qsl = slice(q * BS, (q + 1) * BS)
                            nc.vector.tensor_tensor(hnew_sb[bt][:, qsl],
                                                    hnew_sb[bt][:, qsl], hatt[:], OP.add)
                            dh = pctmp.tile([128, BS], f32, tag="dhq", name="dhq")
                            nc.vector.tensor_tensor(dh[:], hnew_sb[bt][:, qsl],
                                                    hx_sb[bt][:, qsl], OP.subtract)
                            ho = pctmp.tile([128, BS], f32, tag="hoq", name="hoq", bufs=4)
                            nc.vector.scalar_tensor_tensor(ho[:], dh[:],
                                                           mask_sb[bt][:, q:q + 1],
                                                           hx_sb[bt][:, qsl], OP.mult, OP.add)
                            nc.sync.dma_start(out=hx_out[bt * 128:(bt + 1) * 128, qsl],
                                              in_=ho[:])

    _install_bir_fix(nc)
    return nc


# ---------------------------------------------------------------------------
# Host wrapper
# ---------------------------------------------------------------------------

def kernel(inp, hx, cx, wq_inp, wk_inp, wv_inp, w_ih, w_hh, b_ih, b_hh,
           wq_c, wk_c, wv_c, fc_w, fc_b, gate_w, gate_b, step=None):
    global last_exec_time_ns, last_results

    inp = np.asarray(inp, np.float32)
    hx = np.asarray(hx, np.float32)
    cx = np.asarray(cx, np.float32)
    wq_inp = np.asarray(wq_inp, np.float32)
    wk_inp = np.asarray(wk_inp, np.float32)
    wv_inp = np.asarray(wv_inp, np.float32)
    w_ih = np.asarray(w_ih, np.float32)
    w_hh = np.asarray(w_hh, np.float32)
    b_ih = np.asarray(b_ih, np.float32)
    b_hh = np.asarray(b_hh, np.float32)
    wq_c = np.asarray(wq_c, np.float32)
    wk_c = np.asarray(wk_c, np.float32)
    wv_c = np.asarray(wv_c, np.float32)
    fc_w = np.asarray(fc_w, np.float32)
    fc_b = np.asarray(fc_b, np.float32)
    gate_w = np.asarray(gate_w, np.float32)
    gate_b = np.asarray(gate_b, np.float32)

    if "nc" not in _CACHE:
        _CACHE["nc"] = _build()
    nc = _CACHE["nc"]

    # shared (replicated) tensors
    # permute gate columns so 512-wide block j holds gate type j%4 for
    # hidden chunk j//4 (matches the device's incremental LSTM evaluation)
    perm = np.concatenate([np.arange(gt * NHID + t * 512, gt * NHID + (t + 1) * 512)
                           for t in range(4) for gt in range(4)])
    wcat = np.concatenate([w_ih.T, w_hh.T], axis=0)[:, perm]
    wcatT = np.ascontiguousarray(wcat).astype(BF16)
    biasc = (b_ih + b_hh)[perm].astype(BF16).reshape(1, GATES)
    shared = {
        "wq": np.ascontiguousarray(wq_inp.reshape(NB, 2, 128, DKI).transpose(2, 1, 0, 3)),
        "wk1": np.ascontiguousarray(wk_inp[1].reshape(8, 128, DKI).transpose(1, 0, 2)),
        "wv1": np.ascontiguousarray(wv_inp[1].reshape(8, 128, BS).transpose(1, 0, 2)),
        "wcatT": wcatT,
        "biasc": biasc,
        "wqc": np.ascontiguousarray(wq_c.astype(BF16).reshape(NB, 2, 128, 128).transpose(2, 1, 0, 3)),
        "wkc": np.ascontiguousarray(wk_c.astype(BF16).reshape(NB, 2, 128, 128).transpose(2, 1, 0, 3)),
        "wvc": np.ascontiguousarray(wv_c.astype(BF16).reshape(NB, 2, 128, 128).transpose(2, 1, 0, 3)),
        "fcw": fc_w.astype(BF16),
        "gw": gate_w.astype(BF16),
        "fcb": fc_b.astype(BF16).reshape(1, BS),
        "gb": gate_b.astype(BF16).reshape(1, BS),
    }

    in_maps = []
    for c in range(NCORES):
        rs = slice(c * BSH, (c + 1) * BSH)
        inpT = inp[rs].T.reshape(8, 128, BSH).transpose(1, 0, 2)
        hxT = hx[rs].T.reshape(16, 128, BSH).transpose(1, 0, 2)
        m = {
            "inpT": np.ascontiguousarray(inpT),
            "hxT_f": np.ascontiguousarray(hxT),
            "hxT_b": np.ascontiguousarray(hxT.astype(BF16)),
            "hx_bm": np.ascontiguousarray(hx[rs]),
            "cx_bm": np.ascontiguousarray(cx[rs]),
        }
        m.update(shared)
        in_maps.append(m)

    from concourse.bass_utils import run_bass_kernel_spmd
    trace = bool(int(os.environ.get("BASS_KTRACE", "0")))
    res = run_bass_kernel_spmd(nc, in_maps, list(range(NCORES)), trace=trace)
    last_exec_time_ns = res.exec_time_ns
    last_results = res

    hx_full = np.empty((B, NHID), np.float32)
    cx_full = np.empty((B, NHID), np.float32)
    mask_full = np.empty((B, NHID), np.float32)
    for c in range(NCORES):
        rs = slice(c * BSH, (c + 1) * BSH)
        hx_full[rs] = res.results[c]["hx_out"]
        cx_full[rs] = res.results[c]["cx_out"]
        mask_full[rs] = res.results[c]["mask_out"]
    return hx_full, cx_full, mask_full


# revision 15
# speedup vs baseline: 1.5562x; 1.3722x over previous
"""Trainium2 Bass kernel for nn_BlocksCore (RIMs BlocksCore step).

Strategy: data-parallel over batch B=2048 across 8 NeuronCores (256 rows
each). All parameters replicated. Per-core computation:

  1. input attention (f32): k1 = inp@wk1, v1 = inp@wv1, q_n = hx_n@wq_n,
     s1[b,n] = q_n.k1 / 8 (zero-slot score is exactly 0, so softmax over
     [0, s1] collapses to sigmoid);  inp_flat[b, n*256+j] = sig(s1)[b,n]*v1[b,j]
  2. top-k mask: keep the 4 blocks with largest s1 (drop the 4 that attend
     most to the null slot), per row
  3. LSTM cell (bf16 matmuls): gates = [inp_flat|hx] @ [w_ih|w_hh]^T + b
  4. communication attention among the 8 blocks (4 heads, dk=dv=32),
     gated residual: hx_new = h + sigmoid(c@gw+gb)*tanh(c@fw+fb)
  5. masked update of hx/cx.

Layout: batch-major ([batch_p, feat]) for elementwise work; matmuls take
pre-transposed (feature-major) activations as stationary operands. Weights
are pre-transposed/cast on the host. Outputs are batch-major, so the host
just concatenates the 8 shards.
"""

import json
import os

import numpy as np
import ml_dtypes

BF16 = ml_dtypes.bfloat16

B = 2048
NCORES = 8
BSH = B // NCORES          # 256 batch rows per core
NINP = 1024
NHID = 2048
NB = 8                     # blocks
BS = 256                   # block size (NHID / NB)
DKI = 64                   # input-attention d_k
GATES = 4 * NHID           # 8192
KX = 2 * NHID              # LSTM contraction: [inp_flat(2048) | hx(2048)]

_CACHE = {}
last_exec_time_ns = None
last_results = None

# ---------------------------------------------------------------------------
# BIR post-fix: this toolchain's core_v3 codegen supports only one sync-wait
# per CTRL-class instruction (Drain/NoOp/branch). Tile's final drain can carry
# several; hoist extras onto single-wait EventSemaphore instructions.
# ---------------------------------------------------------------------------
# applies to every opcode in this build, so split waits on all of them


def _fix_bir_json(bir_bytes: bytes) -> bytes:
    bir = json.loads(bir_bytes)
    for fn in bir.get("functions", []):
        for blk in fn.get("blocks", []):
            out = []
            for ins in blk.get("instructions", []):
                si = ins.get("sync_info") or {}
                waits = si.get("on_wait") or []
                if len(waits) > 1:
                    for j, w in enumerate(waits[:-1]):
                        out.append({
                            "name": f"{ins['name']}-w{j}",
                            "engine": ins["engine"],
                            "opcode": "EventSemaphore",
                            "ins": [],
                            "outs": [],
                            "sync_info": {"on_update": [], "on_wait": [w]},
                        })
                    si = dict(si)
                    si["on_wait"] = [waits[-1]]
                    ins = dict(ins)
                    ins["sync_info"] = si
                out.append(ins)
            blk["instructions"] = out
    return json.dumps(bir).encode()


def _install_bir_fix(nc):
    orig = nc.to_json_bytes

    def patched(*a, **k):
        return _fix_bir_json(orig(*a, **k))

    nc.to_json_bytes = patched


# ---------------------------------------------------------------------------
# Device kernel
# ---------------------------------------------------------------------------

def _build():
    import concourse.bass as bass
    import concourse.tile as tile
    from concourse import mybir

    f32 = mybir.dt.float32
    bf16 = mybir.dt.bfloat16
    OP = mybir.AluOpType
    AF = mybir.ActivationFunctionType
    AX = mybir.AxisListType

    nc = bass.Bass()

    # ---- I/O ------------------------------------------------------------
    inpT = nc.declare_dram_parameter("inpT", [128, 8, BSH], f32, isOutput=False)
    hxT_f = nc.declare_dram_parameter("hxT_f", [128, 16, BSH], f32, isOutput=False)
    hxT_b = nc.declare_dram_parameter("hxT_b", [128, 16, BSH], bf16, isOutput=False)
    hx_bm = nc.declare_dram_parameter("hx_bm", [BSH, NHID], f32, isOutput=False)
    cx_bm = nc.declare_dram_parameter("cx_bm", [BSH, NHID], f32, isOutput=False)
    wq = nc.declare_dram_parameter("wq", [128, 2, NB, DKI], f32, isOutput=False)
    wk1 = nc.declare_dram_parameter("wk1", [128, 8, DKI], f32, isOutput=False)
    wv1 = nc.declare_dram_parameter("wv1", [128, 8, BS], f32, isOutput=False)
    wcatT = nc.declare_dram_parameter("wcatT", [KX, GATES], bf16, isOutput=False)
    biasc = nc.declare_dram_parameter("biasc", [1, GATES], bf16, isOutput=False)
    wqc = nc.declare_dram_parameter("wqc", [128, 2, NB, 128], bf16, isOutput=False)
    wkc = nc.declare_dram_parameter("wkc", [128, 2, NB, 128], bf16, isOutput=False)
    wvc = nc.declare_dram_parameter("wvc", [128, 2, NB, 128], bf16, isOutput=False)
    fcw = nc.declare_dram_parameter("fcw", [128, BS], bf16, isOutput=False)
    gw = nc.declare_dram_parameter("gw", [128, BS], bf16, isOutput=False)
    fcb = nc.declare_dram_parameter("fcb", [1, BS], bf16, isOutput=False)
    gb = nc.declare_dram_parameter("gb", [1, BS], bf16, isOutput=False)
    hx_out = nc.declare_dram_parameter("hx_out", [BSH, NHID], f32, isOutput=True)
    cx_out = nc.declare_dram_parameter("cx_out", [BSH, NHID], f32, isOutput=True)
    mask_out = nc.declare_dram_parameter("mask_out", [BSH, NHID], f32, isOutput=True)

    # ---- inline constants ----------------------------------------------
    ident_np = np.eye(128, dtype=BF16)
    # score-placement selector: for query block q, out row m = h*8+q gets the
    # head-h sum of a [128]-feature product vector (d -> h = d//32)
    hq_np = np.zeros((128, NB, 32), dtype=BF16)
    for d in range(128):
        for q in range(NB):
            hq_np[d, q, (d // 32) * 8 + q] = 1
    # head expander: for query block q, out feature m (=h*32+d) reads score
    # row r = (m//32)*8 + q
    e32_np = np.zeros((32, NB, 128), dtype=BF16)
    for m in range(128):
        for q in range(NB):
            e32_np[(m // 32) * 8 + q, q, m] = 1
    identb = nc.inline_tensor(ident_np, "identb")
    hqc = nc.inline_tensor(hq_np, "hqc")
    e32b = nc.inline_tensor(e32_np, "e32b")
    e32f = nc.inline_tensor(e32_np.astype(np.float32), "e32f")
    ones1c = nc.inline_tensor(np.ones((1, 128), dtype=BF16), "ones1c")

    with tile.TileContext(nc) as tc:
        with tc.tile_pool(name="cp", bufs=1) as cp, \
             tc.tile_pool(name="pp", bufs=1) as pp:
            # constants to SBUF
            identb_sb = cp.tile([128, 128], bf16)
            nc.sync.dma_start(out=identb_sb[:], in_=identb[:])
            hq_sb = cp.tile([128, NB, 32], bf16)
            nc.sync.dma_start(out=hq_sb[:], in_=hqc[:])
            e32b_sb = cp.tile([32, NB, 128], bf16)
            nc.sync.dma_start(out=e32b_sb[:], in_=e32b[:])
            e32f_sb = cp.tile([32, NB, 128], f32)
            nc.sync.dma_start(out=e32f_sb[:], in_=e32f[:])
            ones1_sb = cp.tile([1, 128], bf16)
            nc.sync.dma_start(out=ones1_sb[:], in_=ones1c[:])
            fcw_sb = cp.tile([128, BS], bf16)
            nc.sync.dma_start(out=fcw_sb[:], in_=fcw[:])
            gw_sb = cp.tile([128, BS], bf16)
            nc.sync.dma_start(out=gw_sb[:], in_=gw[:])
            fcb_sb = cp.tile([1, BS], bf16)
            nc.sync.dma_start(out=fcb_sb[:], in_=fcb[:])
            gb_sb = cp.tile([1, BS], bf16)
            nc.sync.dma_start(out=gb_sb[:], in_=gb[:])
            ones256_sb = cp.tile([128, BS], f32)
            nc.vector.memset(ones256_sb[:], 1.0)

            # persistent inputs / intermediates
            hxTb_sb = pp.tile([128, 16, BSH], bf16)
            nc.sync.dma_start(out=hxTb_sb[:], in_=hxT_b[:])
            hx_sb = [pp.tile([128, NHID], f32, tag=f"hx{bt}", name=f"hx{bt}") for bt in range(2)]
            cx_sb = [pp.tile([128, NHID], f32, tag=f"cx{bt}", name=f"cx{bt}") for bt in range(2)]

            xt_sb = pp.tile([128, 16, 2, 128], bf16)      # inp_flat^T tiles
            hnew_sb = [pp.tile([128, NHID], f32, tag=f"hn{bt}", name=f"hn{bt}") for bt in range(2)]
            cnew_sb = [pp.tile([128, NHID], f32, tag=f"cn{bt}", name=f"cn{bt}") for bt in range(2)]
            hnewT_sb = pp.tile([128, 16, BSH], bf16)
            mask_sb = [pp.tile([128, NB], f32, tag=f"mk{bt}", name=f"mk{bt}") for bt in range(2)]
            sig_sb = [pp.tile([128, NB], f32, tag=f"sg{bt}", name=f"sg{bt}") for bt in range(2)]

            # ============================ phase A ========================
            with tc.tile_pool(name="pa", bufs=1) as pa, \
                 tc.tile_pool(name="pa2", bufs=2) as pa2, \
                 tc.tile_pool(name="paps", bufs=2, space="PSUM") as paps, \
                 tc.tile_pool(name="patp", bufs=2, space="PSUM") as patp:
                inpT_sb = pa.tile([128, 8, BSH], f32)
                nc.scalar.dma_start(out=inpT_sb[:], in_=inpT[:])
                hxTf_sb = pa.tile([128, 16, BSH], f32)
                nc.scalar.dma_start(out=hxTf_sb[:], in_=hxT_f[:])
                wk1_sb = pa.tile([128, 8, DKI], f32)
                nc.sync.dma_start(out=wk1_sb[:], in_=wk1[:])
                wv1_sb = pa.tile([128, 8, BS], f32)
                nc.sync.dma_start(out=wv1_sb[:], in_=wv1[:])
                wq_sb = pa.tile([128, 2, NB, DKI], f32)
                nc.sync.dma_start(out=wq_sb[:], in_=wq[:])

                for bt in range(2):
                    bsl = slice(bt * 128, (bt + 1) * 128)
                    k1_ps = paps.tile([128, DKI], f32, tag="k1")
                    for k in range(8):
                        nc.tensor.matmul(k1_ps[:], inpT_sb[:, k, bsl], wk1_sb[:, k, :],
                                         start=(k == 0), stop=(k == 7))
                    k1s = pa2.tile([128, DKI], f32, tag="k1s")
                    nc.vector.tensor_copy(k1s[:], k1_ps[:])

                    v1_ps = paps.tile([128, BS], f32, tag="v1")
                    for k in range(8):
                        nc.tensor.matmul(v1_ps[:], inpT_sb[:, k, bsl], wv1_sb[:, k, :],
                                         start=(k == 0), stop=(k == 7))
                    v1s = pa2.tile([128, BS], f32, tag="v1s")
                    nc.vector.tensor_copy(v1s[:], v1_ps[:])

                    q_ps = paps.tile([128, NB, DKI], f32, tag="q")
                    for n in range(NB):
                        for s in range(2):
                            nc.tensor.matmul(q_ps[:, n, :],
                                             hxTf_sb[:, 2 * n + s, bsl],
                                             wq_sb[:, s, n, :],
                                             start=(s == 0), stop=(s == 1))
                    prod = pa2.tile([128, NB, DKI], f32, tag="prod")
                    for n in range(NB):
                        nc.vector.tensor_tensor(prod[:, n, :], q_ps[:, n, :], k1s[:], OP.mult)
                    s1 = pa2.tile([128, NB], f32, tag="s1")
                    nc.vector.reduce_sum(s1[:], prod[:], axis=AX.X)
                    nc.scalar.activation(sig_sb[bt][:], s1[:], AF.Sigmoid, scale=0.125)

                    # top-4 mask: keep blocks whose s1 is among the 4 largest
                    cnt = pa2.tile([128, NB], f32, tag="cnt")
                    tmp = pa2.tile([128, NB], f32, tag="tmp")
                    for n in range(NB):
                        nc.vector.tensor_single_scalar(tmp[:], s1[:], s1[:, n:n + 1], OP.is_gt)
                        nc.vector.reduce_sum(cnt[:, n:n + 1], tmp[:], axis=AX.X)
                    nc.vector.tensor_single_scalar(mask_sb[bt][:], cnt[:], 4.0, OP.is_lt)

                    # inp_flat (batch-major, bf16) then transpose to xt tiles
                    ifl = pa2.tile([128, NB, BS], bf16, tag="ifl")
                    for n in range(NB):
                        nc.vector.tensor_single_scalar(ifl[:, n, :], v1s[:],
                                                       sig_sb[bt][:, n:n + 1], OP.mult)
                    for ft in range(16):
                        tp = patp.tile([128, 128], bf16, tag="tp")
                        nc.tensor.transpose(tp[:], ifl[:, ft // 2, (ft % 2) * 128:(ft % 2) * 128 + 128],
                                            identb_sb[:])
                        nc.scalar.copy(xt_sb[:, ft, bt, :], tp[:])

            # ============================ phase B ========================
            # wcatT/biasc columns are host-permuted: 512-wide block j holds
            # gate type j%4 (0=i,1=f,2=g,3=o) for hidden chunk j//4. One
            # 2048-col group = all four gates of one hidden chunk, so the
            # LSTM elementwise runs incrementally per group. The hx half of
            # the contraction (k>=16) runs first so PE can start before the
            # input-attention phase finishes producing inp_flat^T.
            with tc.tile_pool(name="pw", bufs=5) as pw, \
                 tc.tile_pool(name="pact", bufs=2) as pact, \
                 tc.tile_pool(name="pb2", bufs=2) as pb2:
                hnb_sb = [pb2.tile([128, NHID], bf16, tag=f"hnb{bt}", name=f"hnb{bt}",
                                   bufs=1) for bt in range(2)]
                for bt in range(2):
                    nc.sync.dma_start(out=cx_sb[bt][:], in_=cx_bm[bt * 128:(bt + 1) * 128, :])
                for bt in range(2):
                    nc.scalar.dma_start(out=hx_sb[bt][:], in_=hx_bm[bt * 128:(bt + 1) * 128, :])
                ks_order = list(range(16, 32)) + list(range(16))
                pbps_cm = tc.tile_pool(name="pbps", bufs=1, space="PSUM")
                pbps = pbps_cm.__enter__()
                for ncpp in range(4):
                    g = {}
                    for bt in range(2):
                        for c in range(4):
                            g[bt, c] = pbps.tile([128, 512], f32, tag=f"g{bt}{c}",
                                                 name=f"g{bt}{c}")
                    for ki, k in enumerate(ks_order):
                        w = pw.tile([128, 2048], bf16)
                        nc.sync.dma_start(out=w[:, 0:1024],
                                          in_=wcatT[k * 128:(k + 1) * 128,
                                                    ncpp * 2048:ncpp * 2048 + 1024])
                        nc.scalar.dma_start(out=w[:, 1024:2048],
                                            in_=wcatT[k * 128:(k + 1) * 128,
                                                      ncpp * 2048 + 1024:(ncpp + 1) * 2048])
                        for bt in range(2):
                            if k < 16:
                                lhsT = xt_sb[:, k, bt, :]
                            else:
                                lhsT = hxTb_sb[:, k - 16, bt * 128:(bt + 1) * 128]
                            for c in range(4):
                                nc.tensor.matmul(g[bt, c][:], lhsT,
                                                 w[:, c * 512:(c + 1) * 512],
                                                 start=(ki == 0), stop=False)
                    bsl_t = pb2.tile([1, 2048], bf16, tag="biasc", name="biascsl", bufs=2)
                    nc.sync.dma_start(out=bsl_t[:],
                                      in_=biasc[:, ncpp * 2048:(ncpp + 1) * 2048])
                    act_cur = {}
                    for bt in range(2):
                        for c in range(4):
                            nc.tensor.matmul(g[bt, c][:], ones1_sb[:],
                                             bsl_t[:, c * 512:(c + 1) * 512],
                                             start=False, stop=True)
                            a = pact.tile([128, 512], f32, tag=f"a{c}{bt}", name=f"a{c}{bt}")
                            func = AF.Tanh if c == 2 else AF.Sigmoid
                            nc.scalar.activation(a[:], g[bt, c][:], func)
                            act_cur[c, bt] = a
                    t = ncpp
                    sl = slice(t * 512, (t + 1) * 512)
                    for bt in range(2):
                        t1 = pb2.tile([128, 512], f32, tag="t1", name="t1")
                        nc.vector.tensor_tensor(t1[:], act_cur[1, bt][:],
                                                cx_sb[bt][:, sl], OP.mult)
                        t2 = pb2.tile([128, 512], f32, tag="t2", name="t2")
                        nc.vector.tensor_tensor(t2[:], act_cur[0, bt][:],
                                                act_cur[2, bt][:], OP.mult)
                        nc.vector.tensor_tensor(cnew_sb[bt][:, sl], t1[:], t2[:], OP.add)
                        t3 = pb2.tile([128, 512], f32, tag="t3", name="t3")
                        nc.scalar.activation(t3[:], cnew_sb[bt][:, sl], AF.Tanh)
                        nc.vector.tensor_tensor(hnew_sb[bt][:, sl], act_cur[3, bt][:],
                                                t3[:], OP.mult)
                        nc.vector.tensor_copy(hnb_sb[bt][:, sl], hnew_sb[bt][:, sl])
                pbps_cm.__exit__(None, None, None)

                # cx/mask outputs no longer depend on anything downstream:
                # emit them now so they overlap the communication attention.
                for bt in range(2):
                    rsl = slice(bt * 128, (bt + 1) * 128)
                    dc = pb2.tile([128, NHID], f32, tag="dc", name="dc", bufs=1)
                    nc.vector.tensor_tensor(dc[:], cnew_sb[bt][:], cx_sb[bt][:], OP.subtract)
                    co = pb2.tile([128, NHID], f32, tag="co", name="co", bufs=1)
                    mo = pb2.tile([128, NHID], f32, tag="mo", name="mo", bufs=1)
                    for n in range(NB):
                        sl = slice(n * BS, (n + 1) * BS)
                        nc.vector.scalar_tensor_tensor(co[:, sl], dc[:, sl],
                                                       mask_sb[bt][:, n:n + 1],
                                                       cx_sb[bt][:, sl], OP.mult, OP.add)
                        nc.scalar.mul(mo[:, sl], ones256_sb[:],
                                      mask_sb[bt][:, n:n + 1])
                    nc.sync.dma_start(out=cx_out[rsl, :], in_=co[:])
                    nc.sync.dma_start(out=mask_out[rsl, :], in_=mo[:])

                with tc.tile_pool(name="pbtp", bufs=3, space="PSUM") as pbtp:
                    for bt in range(2):
                        for ft in range(16):
                            tp = pbtp.tile([128, 128], bf16, tag="tp2", name="tp2")
                            nc.tensor.transpose(tp[:], hnb_sb[bt][:, ft * 128:(ft + 1) * 128],
                                                identb_sb[:])
                            nc.scalar.copy(hnewT_sb[:, ft, bt * 128:(bt + 1) * 128], tp[:])

            # ============================ phase C ========================
            with tc.tile_pool(name="pcw", bufs=1) as pcw, \
                 tc.tile_pool(name="pctmp", bufs=2) as pctmp:
                qc_sb = pcw.tile([128, NB, BSH], bf16)
                kc_sb = pcw.tile([128, NB, BSH], bf16)
                vc_sb = pcw.tile([128, NB, BSH], bf16)
                exp_sb = pcw.tile([32, NB, BSH], bf16)
                recip_sb = pcw.tile([32, BSH], f32)
                coutb_sb = pcw.tile([128, NB, BSH], bf16)
                wqc_sb = pcw.tile([128, 2, NB, 128], bf16)
                nc.sync.dma_start(out=wqc_sb[:], in_=wqc[:])
                wkc_sb = pcw.tile([128, 2, NB, 128], bf16)
                nc.sync.dma_start(out=wkc_sb[:], in_=wkc[:])
                wvc_sb = pcw.tile([128, 2, NB, 128], bf16)
                nc.sync.dma_start(out=wvc_sb[:], in_=wvc[:])

                with tc.tile_pool(name="pcp1", bufs=2, space="PSUM") as pcp1:
                    for n in range(NB):
                        for wsb, dst in ((wqc_sb, qc_sb), (wkc_sb, kc_sb), (wvc_sb, vc_sb)):
                            ps = pcp1.tile([128, BSH], f32, tag="proj")
                            for s in range(2):
                                nc.tensor.matmul(ps[:], wsb[:, s, n, :],
                                                 hnewT_sb[:, 2 * n + s, :],
                                                 start=(s == 0), stop=(s == 1))
                            nc.scalar.copy(dst[:, n, :], ps[:])

                with tc.tile_pool(name="psS", bufs=1, space="PSUM") as psS:
                    S = psS.tile([32, NB, BSH], f32)
                    for k in range(NB):
                        for q in range(NB):
                            pr = pctmp.tile([128, BSH], bf16, tag="pr")
                            nc.vector.tensor_tensor(pr[:], qc_sb[:, q, :], kc_sb[:, k, :], OP.mult)
                            nc.tensor.matmul(S[:, k, :], hq_sb[:, q, :], pr[:],
                                             start=(q == 0), stop=(q == 7))
                    nc.scalar.activation(exp_sb[:], S[:], AF.Exp,
                                         scale=float(1.0 / np.sqrt(32.0)))
                    denom = pctmp.tile([32, BSH], f32, tag="denom")
                    nc.vector.reduce_sum(denom[:], exp_sb[:].rearrange("p k b -> p b k"),
                                         axis=AX.X)
                    nc.vector.reciprocal(recip_sb[:], denom[:])

                with tc.tile_pool(name="psU", bufs=1, space="PSUM") as psU, \
                     tc.tile_pool(name="psRE", bufs=2, space="PSUM") as psRE:
                    for q in range(NB):
                        U = psU.tile([128, NB, BSH], f32, tag="U")
                        for kp in range(4):
                            nc.tensor.matmul(U[:, 2 * kp:2 * kp + 2, :], e32b_sb[:, q, :],
                                             exp_sb[:, 2 * kp:2 * kp + 2, :],
                                             start=True, stop=True)
                        prods = pctmp.tile([128, NB, BSH], f32, tag="prods")
                        nc.vector.tensor_tensor(prods[:], U[:], vc_sb[:], OP.mult)
                        raw = pctmp.tile([128, BSH], f32, tag="raw")
                        nc.vector.reduce_sum(raw[:], prods[:].rearrange("p k b -> p b k"),
                                             axis=AX.X)
                        RE = psRE.tile([128, BSH], f32, tag="RE")
                        nc.tensor.matmul(RE[:], e32f_sb[:, q, :], recip_sb[:],
                                         start=True, stop=True)
                        nc.vector.tensor_tensor(coutb_sb[:, q, :], raw[:], RE[:], OP.mult)

                with tc.tile_pool(name="psOG", bufs=2, space="PSUM") as psOG:
                    for q in range(NB):
                        for bt in range(2):
                            csl = coutb_sb[:, q, bt * 128:(bt + 1) * 128]
                            ops_ = psOG.tile([128, BS], f32, tag="o")
                            nc.tensor.matmul(ops_[:], csl, fcw_sb[:], start=True, stop=False)
                            nc.tensor.matmul(ops_[:], ones1_sb[:], fcb_sb[:], start=False, stop=True)
                            gps_ = psOG.tile([128, BS], f32, tag="gg")
                            nc.tensor.matmul(gps_[:], csl, gw_sb[:], start=True, stop=False)
                            nc.tensor.matmul(gps_[:], ones1_sb[:], gb_sb[:], start=False, stop=True)
                            tano = pctmp.tile([128, BS], f32, tag="tano")
                            nc.scalar.activation(tano[:], ops_[:], AF.Tanh)
                            sg = pctmp.tile([128, BS], f32, tag="sgx")
                            nc.scalar.activation(sg[:], gps_[:], AF.Sigmoid)
                            hatt = pctmp.tile([128, BS], f32, tag="hatt")
                            nc.vector.tensor_tensor(hatt[:], sg[:], tano[:], OP.mult)
                            qsl = slice(q * BS, (q + 1) * BS)
                            nc.vector.tensor_tensor(hnew_sb[bt][:, qsl],
                                                    hnew_sb[bt][:, qsl], hatt[:], OP.add)
                            dh = pctmp.tile([128, BS], f32, tag="dhq", name="dhq")
                            nc.vector.tensor_tensor(dh[:], hnew_sb[bt][:, qsl],
                                                    hx_sb[bt][:, qsl], OP.subtract)
                            ho = pctmp.tile([128, BS], f32, tag="hoq", name="hoq", bufs=4)
                            nc.vector.scalar_tensor_tensor(ho[:], dh[:],
                                                           mask_sb[bt][:, q:q + 1],
                                                           hx_sb[bt][:, qsl], OP.mult, OP.add)
                            nc.sync.dma_start(out=hx_out[bt * 128:(bt + 1) * 128, qsl],
                                              in_=ho[:])

    _install_bir_fix(nc)
    return nc


# ---------------------------------------------------------------------------
# Host wrapper
# ---------------------------------------------------------------------------

def kernel(inp, hx, cx, wq_inp, wk_inp, wv_inp, w_ih, w_hh, b_ih, b_hh,
           wq_c, wk_c, wv_c, fc_w, fc_b, gate_w, gate_b, step=None):
    global last_exec_time_ns, last_results

    inp = np.asarray(inp, np.float32)
    hx = np.asarray(hx, np.float32)
    cx = np.asarray(cx, np.float32)
    wq_inp = np.asarray(wq_inp, np.float32)
    wk_inp = np.asarray(wk_inp, np.float32)
    wv_inp = np.asarray(wv_inp, np.float32)
    w_ih = np.asarray(w_ih, np.float32)
    w_hh = np.asarray(w_hh, np.float32)
    b_ih = np.asarray(b_ih, np.float32)
    b_hh = np.asarray(b_hh, np.float32)
    wq_c = np.asarray(wq_c, np.float32)
    wk_c = np.asarray(wk_c, np.float32)
    wv_c = np.asarray(wv_c, np.float32)
    fc_w = np.asarray(fc_w, np.float32)
    fc_b = np.asarray(fc_b, np.float32)
    gate_w = np.asarray(gate_w, np.float32)
    gate_b = np.asarray(gate_b, np.float32)

    if "nc" not in _CACHE:
        _CACHE["nc"] = _build()
    nc = _CACHE["nc"]

    # shared (replicated) tensors
    # permute gate columns so 512-wide block j holds gate type j%4 for
    # hidden chunk j//4 (matches the device's incremental LSTM evaluation)
    perm = np.concatenate([np.arange(gt * NHID + t * 512, gt * NHID + (t + 1) * 512)
                           for t in range(4) for gt in range(4)])
    wcat = np.concatenate([w_ih.T, w_hh.T], axis=0)[:, perm]
    wcatT = np.ascontiguousarray(wcat).astype(BF16)
    biasc = (b_ih + b_hh)[perm].astype(BF16).reshape(1, GATES)
    shared = {
        "wq": np.ascontiguousarray(wq_inp.reshape(NB, 2, 128, DKI).transpose(2, 1, 0, 3)),
        "wk1": np.ascontiguousarray(wk_inp[1].reshape(8, 128, DKI).transpose(1, 0, 2)),
        "wv1": np.ascontiguousarray(wv_inp[1].reshape(8, 128, BS).transpose(1, 0, 2)),
        "wcatT": wcatT,
        "biasc": biasc,
        "wqc": np.ascontiguousarray(wq_c.astype(BF16).reshape(NB, 2, 128, 128).transpose(2, 1, 0, 3)),
        "wkc": np.ascontiguousarray(wk_c.astype(BF16).reshape(NB, 2, 128, 128).transpose(2, 1, 0, 3)),
        "wvc": np.ascontiguousarray(wv_c.astype(BF16).reshape(NB, 2, 128, 128).transpose(2, 1, 0, 3)),
        "fcw": fc_w.astype(BF16),
        "gw": gate_w.astype(BF16),
        "fcb": fc_b.astype(BF16).reshape(1, BS),
        "gb": gate_b.astype(BF16).reshape(1, BS),
    }

    in_maps = []
    for c in range(NCORES):
        rs = slice(c * BSH, (c + 1) * BSH)
        inpT = inp[rs].T.reshape(8, 128, BSH).transpose(1, 0, 2)
        hxT = hx[rs].T.reshape(16, 128, BSH).transpose(1, 0, 2)
        m = {
            "inpT": np.ascontiguousarray(inpT),
            "hxT_f": np.ascontiguousarray(hxT),
            "hxT_b": np.ascontiguousarray(hxT.astype(BF16)),
            "hx_bm": np.ascontiguousarray(hx[rs]),
            "cx_bm": np.ascontiguousarray(cx[rs]),
        }
        m.update(shared)
        in_maps.append(m)

    from concourse.bass_utils import run_bass_kernel_spmd
    trace = bool(int(os.environ.get("BASS_KTRACE", "0")))
    res = run_bass_kernel_spmd(nc, in_maps, list(range(NCORES)), trace=trace)
    last_exec_time_ns = res.exec_time_ns
    last_results = res

    hx_full = np.empty((B, NHID), np.float32)
    cx_full = np.empty((B, NHID), np.float32)
    mask_full = np.empty((B, NHID), np.float32)
    for c in range(NCORES):
        rs = slice(c * BSH, (c + 1) * BSH)
        hx_full[rs] = res.results[c]["hx_out"]
        cx_full[rs] = res.results[c]["cx_out"]
        mask_full[rs] = res.results[c]["mask_out"]
    return hx_full, cx_full, mask_full


# revision 17
# speedup vs baseline: 1.6436x; 1.0562x over previous
"""Trainium2 Bass kernel for nn_BlocksCore (RIMs BlocksCore step).

Strategy: data-parallel over batch B=2048 across 8 NeuronCores (256 rows
each). All parameters replicated. Per-core computation:

  1. input attention (f32): k1 = inp@wk1, v1 = inp@wv1, q_n = hx_n@wq_n,
     s1[b,n] = q_n.k1 / 8 (zero-slot score is exactly 0, so softmax over
     [0, s1] collapses to sigmoid);  inp_flat[b, n*256+j] = sig(s1)[b,n]*v1[b,j]
  2. top-k mask: keep the 4 blocks with largest s1 (drop the 4 that attend
     most to the null slot), per row
  3. LSTM cell (bf16 matmuls): gates = [inp_flat|hx] @ [w_ih|w_hh]^T + b
  4. communication attention among the 8 blocks (4 heads, dk=dv=32),
     gated residual: hx_new = h + sigmoid(c@gw+gb)*tanh(c@fw+fb)
  5. masked update of hx/cx.

Layout: batch-major ([batch_p, feat]) for elementwise work; matmuls take
pre-transposed (feature-major) activations as stationary operands. Weights
are pre-transposed/cast on the host. Outputs are batch-major, so the host
just concatenates the 8 shards.
"""

import json
import os

import numpy as np
import ml_dtypes

BF16 = ml_dtypes.bfloat16

B = 2048
NCORES = 8
BSH = B // NCORES          # 256 batch rows per core
NINP = 1024
NHID = 2048
NB = 8                     # blocks
BS = 256                   # block size (NHID / NB)
DKI = 64                   # input-attention d_k
GATES = 4 * NHID           # 8192
KX = 2 * NHID              # LSTM contraction: [inp_flat(2048) | hx(2048)]

_CACHE = {}
last_exec_time_ns = None
last_results = None

# ---------------------------------------------------------------------------
# BIR post-fix: this toolchain's core_v3 codegen supports only one sync-wait
# per CTRL-class instruction (Drain/NoOp/branch). Tile's final drain can carry
# several; hoist extras onto single-wait EventSemaphore instructions.
# ---------------------------------------------------------------------------
# applies to every opcode in this build, so split waits on all of them


def _fix_bir_json(bir_bytes: bytes) -> bytes:
    bir = json.loads(bir_bytes)
    for fn in bir.get("functions", []):
        for blk in fn.get("blocks", []):
            out = []
            for ins in blk.get("instructions", []):
                si = ins.get("sync_info") or {}
                waits = si.get("on_wait") or []
                if len(waits) > 1:
                    for j, w in enumerate(waits[:-1]):
                        out.append({
                            "name": f"{ins['name']}-w{j}",
                            "engine": ins["engine"],
                            "opcode": "EventSemaphore",
                            "ins": [],
                            "outs": [],
                            "sync_info": {"on_update": [], "on_wait": [w]},
                        })
                    si = dict(si)
                    si["on_wait"] = [waits[-1]]
                    ins = dict(ins)
                    ins["sync_info"] = si
                out.append(ins)
            blk["instructions"] = out
    return json.dumps(bir).encode()


def _install_bir_fix(nc):
    orig = nc.to_json_bytes

    def patched(*a, **k):
        return _fix_bir_json(orig(*a, **k))

    nc.to_json_bytes = patched


# ---------------------------------------------------------------------------
# Device kernel
# ---------------------------------------------------------------------------

def _build():
    import concourse.bass as bass
    import concourse.tile as tile
    from concourse import mybir

    f32 = mybir.dt.float32
    bf16 = mybir.dt.bfloat16
    OP = mybir.AluOpType
    AF = mybir.ActivationFunctionType
    AX = mybir.AxisListType

    nc = bass.Bass()

    # ---- I/O ------------------------------------------------------------
    inpT = nc.declare_dram_parameter("inpT", [128, 8, BSH], f32, isOutput=False)
    hxT_f = nc.declare_dram_parameter("hxT_f", [128, 16, BSH], f32, isOutput=False)
    hxT_b = nc.declare_dram_parameter("hxT_b", [128, 16, BSH], bf16, isOutput=False)
    hx_bm = nc.declare_dram_parameter("hx_bm", [BSH, NHID], f32, isOutput=False)
    cx_bm = nc.declare_dram_parameter("cx_bm", [BSH, NHID], f32, isOutput=False)
    wq = nc.declare_dram_parameter("wq", [128, 2, NB, DKI], f32, isOutput=False)
    wk1 = nc.declare_dram_parameter("wk1", [128, 8, DKI], f32, isOutput=False)
    wv1 = nc.declare_dram_parameter("wv1", [128, 8, BS], f32, isOutput=False)
    wcatT = nc.declare_dram_parameter("wcatT", [KX, GATES], bf16, isOutput=False)
    biasc = nc.declare_dram_parameter("biasc", [1, GATES], bf16, isOutput=False)
    wqc = nc.declare_dram_parameter("wqc", [128, 2, NB, 128], bf16, isOutput=False)
    wkc = nc.declare_dram_parameter("wkc", [128, 2, NB, 128], bf16, isOutput=False)
    wvc = nc.declare_dram_parameter("wvc", [128, 2, NB, 128], bf16, isOutput=False)
    fcw = nc.declare_dram_parameter("fcw", [128, BS], bf16, isOutput=False)
    gw = nc.declare_dram_parameter("gw", [128, BS], bf16, isOutput=False)
    fcb = nc.declare_dram_parameter("fcb", [1, BS], bf16, isOutput=False)
    gb = nc.declare_dram_parameter("gb", [1, BS], bf16, isOutput=False)
    hx_out = nc.declare_dram_parameter("hx_out", [BSH, NHID], f32, isOutput=True)
    cx_out = nc.declare_dram_parameter("cx_out", [BSH, NHID], f32, isOutput=True)
    mask_out = nc.declare_dram_parameter("mask_out", [BSH, NHID], f32, isOutput=True)

    # ---- inline constants ----------------------------------------------
    ident_np = np.eye(128, dtype=BF16)
    # score-placement selector: for query block q, out row m = h*8+q gets the
    # head-h sum of a [128]-feature product vector (d -> h = d//32)
    hq_np = np.zeros((128, NB, 32), dtype=BF16)
    for d in range(128):
        for q in range(NB):
            hq_np[d, q, (d // 32) * 8 + q] = 1
    # head expander: for query block q, out feature m (=h*32+d) reads score
    # row r = (m//32)*8 + q
    e32_np = np.zeros((32, NB, 128), dtype=BF16)
    for m in range(128):
        for q in range(NB):
            e32_np[(m // 32) * 8 + q, q, m] = 1
    identb = nc.inline_tensor(ident_np, "identb")
    hqc = nc.inline_tensor(hq_np, "hqc")
    e32b = nc.inline_tensor(e32_np, "e32b")
    e32f = nc.inline_tensor(e32_np.astype(np.float32), "e32f")
    ones1c = nc.inline_tensor(np.ones((1, 128), dtype=BF16), "ones1c")

    with tile.TileContext(nc) as tc:
        with tc.tile_pool(name="cp", bufs=1) as cp, \
             tc.tile_pool(name="pp", bufs=1) as pp:
            # constants to SBUF
            identb_sb = cp.tile([128, 128], bf16)
            nc.sync.dma_start(out=identb_sb[:], in_=identb[:])
            hq_sb = cp.tile([128, NB, 32], bf16)
            nc.sync.dma_start(out=hq_sb[:], in_=hqc[:])
            e32b_sb = cp.tile([32, NB, 128], bf16)
            nc.sync.dma_start(out=e32b_sb[:], in_=e32b[:])
            e32f_sb = cp.tile([32, NB, 128], f32)
            nc.sync.dma_start(out=e32f_sb[:], in_=e32f[:])
            ones1_sb = cp.tile([1, 128], bf16)
            nc.sync.dma_start(out=ones1_sb[:], in_=ones1c[:])
            fcw_sb = cp.tile([128, BS], bf16)
            nc.sync.dma_start(out=fcw_sb[:], in_=fcw[:])
            gw_sb = cp.tile([128, BS], bf16)
            nc.sync.dma_start(out=gw_sb[:], in_=gw[:])
            fcb_sb = cp.tile([1, BS], bf16)
            nc.sync.dma_start(out=fcb_sb[:], in_=fcb[:])
            gb_sb = cp.tile([1, BS], bf16)
            nc.sync.dma_start(out=gb_sb[:], in_=gb[:])
            ones256_sb = cp.tile([128, BS], f32)
            nc.vector.memset(ones256_sb[:], 1.0)

            # persistent inputs / intermediates
            hxTb_sb = pp.tile([128, 16, BSH], bf16)
            nc.sync.dma_start(out=hxTb_sb[:], in_=hxT_b[:])
            hx_sb = [pp.tile([128, NHID], f32, tag=f"hx{bt}", name=f"hx{bt}") for bt in range(2)]
            cx_sb = [pp.tile([128, NHID], f32, tag=f"cx{bt}", name=f"cx{bt}") for bt in range(2)]

            xt_sb = pp.tile([128, 16, 2, 128], bf16)      # inp_flat^T tiles
            hnew_sb = [pp.tile([128, NHID], f32, tag=f"hn{bt}", name=f"hn{bt}") for bt in range(2)]
            cnew_sb = [pp.tile([128, NHID], f32, tag=f"cn{bt}", name=f"cn{bt}") for bt in range(2)]
            hnewT_sb = pp.tile([128, 16, BSH], bf16)
            mask_sb = [pp.tile([128, NB], f32, tag=f"mk{bt}", name=f"mk{bt}") for bt in range(2)]
            sig_sb = [pp.tile([128, NB], f32, tag=f"sg{bt}", name=f"sg{bt}") for bt in range(2)]

            # ============================ phase A ========================
            with tc.tile_pool(name="pa", bufs=1) as pa, \
                 tc.tile_pool(name="pa2", bufs=2) as pa2, \
                 tc.tile_pool(name="paps", bufs=2, space="PSUM") as paps, \
                 tc.tile_pool(name="patp", bufs=2, space="PSUM") as patp:
                inpT_sb = pa.tile([128, 8, BSH], f32)
                nc.scalar.dma_start(out=inpT_sb[:], in_=inpT[:])
                hxTf_sb = pa.tile([128, 16, BSH], f32)
                nc.scalar.dma_start(out=hxTf_sb[:], in_=hxT_f[:])
                wk1_sb = pa.tile([128, 8, DKI], f32)
                nc.sync.dma_start(out=wk1_sb[:], in_=wk1[:])
                wv1_sb = pa.tile([128, 8, BS], f32)
                nc.sync.dma_start(out=wv1_sb[:], in_=wv1[:])
                wq_sb = pa.tile([128, 2, NB, DKI], f32)
                nc.sync.dma_start(out=wq_sb[:], in_=wq[:])

                for bt in range(2):
                    bsl = slice(bt * 128, (bt + 1) * 128)
                    k1_ps = paps.tile([128, DKI], f32, tag="k1")
                    for k in range(8):
                        nc.tensor.matmul(k1_ps[:], inpT_sb[:, k, bsl], wk1_sb[:, k, :],
                                         start=(k == 0), stop=(k == 7))
                    k1s = pa2.tile([128, DKI], f32, tag="k1s")
                    nc.vector.tensor_copy(k1s[:], k1_ps[:])

                    v1_ps = paps.tile([128, BS], f32, tag="v1")
                    for k in range(8):
                        nc.tensor.matmul(v1_ps[:], inpT_sb[:, k, bsl], wv1_sb[:, k, :],
                                         start=(k == 0), stop=(k == 7))
                    v1s = pa2.tile([128, BS], f32, tag="v1s")
                    nc.vector.tensor_copy(v1s[:], v1_ps[:])

                    q_ps = paps.tile([128, NB, DKI], f32, tag="q")
                    for n in range(NB):
                        for s in range(2):
                            nc.tensor.matmul(q_ps[:, n, :],
                                             hxTf_sb[:, 2 * n + s, bsl],
                                             wq_sb[:, s, n, :],
                                             start=(s == 0), stop=(s == 1))
                    prod = pa2.tile([128, NB, DKI], f32, tag="prod")
                    k1a = k1s[:]
                    k1bc = bass.AP(tensor=k1a.tensor, offset=k1a.offset,
                                   ap=[k1a.ap[0], [0, NB], k1a.ap[1]])
                    nc.vector.tensor_tensor(prod[:], q_ps[:], k1bc, OP.mult)
                    s1 = pa2.tile([128, NB], f32, tag="s1")
                    nc.vector.reduce_sum(s1[:], prod[:], axis=AX.X)
                    nc.scalar.activation(sig_sb[bt][:], s1[:], AF.Sigmoid, scale=0.125)

                    # top-4 mask: keep blocks whose s1 is among the 4 largest
                    cnt = pa2.tile([128, NB], f32, tag="cnt")
                    tmp = pa2.tile([128, NB], f32, tag="tmp")
                    for n in range(NB):
                        nc.vector.tensor_single_scalar(tmp[:], s1[:], s1[:, n:n + 1], OP.is_gt)
                        nc.vector.reduce_sum(cnt[:, n:n + 1], tmp[:], axis=AX.X)
                    nc.vector.tensor_single_scalar(mask_sb[bt][:], cnt[:], 4.0, OP.is_lt)

                    # inp_flat (batch-major, bf16) then transpose to xt tiles
                    ifl = pa2.tile([128, NB, BS], bf16, tag="ifl")
                    for n in range(NB):
                        nc.vector.tensor_single_scalar(ifl[:, n, :], v1s[:],
                                                       sig_sb[bt][:, n:n + 1], OP.mult)
                    for ft in range(16):
                        tp = patp.tile([128, 128], bf16, tag="tp")
                        nc.tensor.transpose(tp[:], ifl[:, ft // 2, (ft % 2) * 128:(ft % 2) * 128 + 128],
                                            identb_sb[:])
                        nc.scalar.copy(xt_sb[:, ft, bt, :], tp[:])

            # ============================ phase B ========================
            # wcatT/biasc columns are host-permuted: 512-wide block j holds
            # gate type j%4 (0=i,1=f,2=g,3=o) for hidden chunk j//4. One
            # 2048-col group = all four gates of one hidden chunk, so the
            # LSTM elementwise runs incrementally per group. The hx half of
            # the contraction (k>=16) runs first so PE can start before the
            # input-attention phase finishes producing inp_flat^T.
            with tc.tile_pool(name="pw", bufs=5) as pw, \
                 tc.tile_pool(name="pact", bufs=2) as pact, \
                 tc.tile_pool(name="pb2", bufs=2) as pb2:
                hnb_sb = [pb2.tile([128, NHID], bf16, tag=f"hnb{bt}", name=f"hnb{bt}",
                                   bufs=1) for bt in range(2)]
                for bt in range(2):
                    nc.sync.dma_start(out=cx_sb[bt][:], in_=cx_bm[bt * 128:(bt + 1) * 128, :])
                for bt in range(2):
                    nc.scalar.dma_start(out=hx_sb[bt][:], in_=hx_bm[bt * 128:(bt + 1) * 128, :])
                ks_order = list(range(16, 32)) + list(range(16))
                pbps_cm = tc.tile_pool(name="pbps", bufs=1, space="PSUM")
                pbps = pbps_cm.__enter__()
                for ncpp in range(4):
                    g = {}
                    for bt in range(2):
                        for c in range(4):
                            g[bt, c] = pbps.tile([128, 512], f32, tag=f"g{bt}{c}",
                                                 name=f"g{bt}{c}")
                    for ki, k in enumerate(ks_order):
                        w = pw.tile([128, 2048], bf16)
                        nc.sync.dma_start(out=w[:, 0:1024],
                                          in_=wcatT[k * 128:(k + 1) * 128,
                                                    ncpp * 2048:ncpp * 2048 + 1024])
                        nc.scalar.dma_start(out=w[:, 1024:2048],
                                            in_=wcatT[k * 128:(k + 1) * 128,
                                                      ncpp * 2048 + 1024:(ncpp + 1) * 2048])
                        for bt in range(2):
                            if k < 16:
                                lhsT = xt_sb[:, k, bt, :]
                            else:
                                lhsT = hxTb_sb[:, k - 16, bt * 128:(bt + 1) * 128]
                            for c in range(4):
                                nc.tensor.matmul(g[bt, c][:], lhsT,
                                                 w[:, c * 512:(c + 1) * 512],
                                                 start=(ki == 0), stop=False)
                    bsl_t = pb2.tile([1, 2048], bf16, tag="biasc", name="biascsl", bufs=2)
                    nc.sync.dma_start(out=bsl_t[:],
                                      in_=biasc[:, ncpp * 2048:(ncpp + 1) * 2048])
                    act_cur = {}
                    for bt in range(2):
                        for c in range(4):
                            nc.tensor.matmul(g[bt, c][:], ones1_sb[:],
                                             bsl_t[:, c * 512:(c + 1) * 512],
                                             start=False, stop=True)
                            a = pact.tile([128, 512], f32, tag=f"a{c}{bt}", name=f"a{c}{bt}")
                            func = AF.Tanh if c == 2 else AF.Sigmoid
                            nc.scalar.activation(a[:], g[bt, c][:], func)
                            act_cur[c, bt] = a
                    t = ncpp
                    sl = slice(t * 512, (t + 1) * 512)
                    for bt in range(2):
                        t1 = pb2.tile([128, 512], f32, tag="t1", name="t1")
                        nc.vector.tensor_tensor(t1[:], act_cur[1, bt][:],
                                                cx_sb[bt][:, sl], OP.mult)
                        t2 = pb2.tile([128, 512], f32, tag="t2", name="t2")
                        nc.vector.tensor_tensor(t2[:], act_cur[0, bt][:],
                                                act_cur[2, bt][:], OP.mult)
                        nc.vector.tensor_tensor(cnew_sb[bt][:, sl], t1[:], t2[:], OP.add)
                        t3 = pb2.tile([128, 512], f32, tag="t3", name="t3")
                        nc.scalar.activation(t3[:], cnew_sb[bt][:, sl], AF.Tanh)
                        nc.vector.tensor_tensor(hnew_sb[bt][:, sl], act_cur[3, bt][:],
                                                t3[:], OP.mult)
                        nc.vector.tensor_copy(hnb_sb[bt][:, sl], hnew_sb[bt][:, sl])
                pbps_cm.__exit__(None, None, None)

                # cx/mask outputs no longer depend on anything downstream:
                # emit them now so they overlap the communication attention.
                for bt in range(2):
                    rsl = slice(bt * 128, (bt + 1) * 128)
                    dc = pb2.tile([128, NHID], f32, tag="dc", name="dc", bufs=1)
                    nc.gpsimd.tensor_tensor(dc[:], cnew_sb[bt][:], cx_sb[bt][:], OP.subtract)
                    co = pb2.tile([128, NHID], f32, tag="co", name="co", bufs=1)
                    mo = pb2.tile([128, NHID], f32, tag="mo", name="mo", bufs=1)
                    for n in range(NB):
                        sl = slice(n * BS, (n + 1) * BS)
                        nc.vector.scalar_tensor_tensor(co[:, sl], dc[:, sl],
                                                       mask_sb[bt][:, n:n + 1],
                                                       cx_sb[bt][:, sl], OP.mult, OP.add)
                        nc.scalar.mul(mo[:, sl], ones256_sb[:],
                                      mask_sb[bt][:, n:n + 1])
                    nc.sync.dma_start(out=cx_out[rsl, :], in_=co[:])
                    nc.sync.dma_start(out=mask_out[rsl, :], in_=mo[:])

                with tc.tile_pool(name="pbtp", bufs=3, space="PSUM") as pbtp:
                    for bt in range(2):
                        for ft in range(16):
                            tp = pbtp.tile([128, 128], bf16, tag="tp2", name="tp2")
                            nc.tensor.transpose(tp[:], hnb_sb[bt][:, ft * 128:(ft + 1) * 128],
                                                identb_sb[:])
                            nc.scalar.copy(hnewT_sb[:, ft, bt * 128:(bt + 1) * 128], tp[:])

            # ============================ phase C ========================
            with tc.tile_pool(name="pcw", bufs=1) as pcw, \
                 tc.tile_pool(name="pctmp", bufs=2) as pctmp:
                qc_sb = pcw.tile([128, NB, BSH], bf16)
                kc_sb = pcw.tile([128, NB, BSH], bf16)
                vc_sb = pcw.tile([128, NB, BSH], bf16)
                exp_sb = pcw.tile([32, NB, BSH], bf16)
                recip_sb = pcw.tile([32, BSH], f32)
                coutb_sb = pcw.tile([128, NB, BSH], bf16)
                wqc_sb = pcw.tile([128, 2, NB, 128], bf16)
                nc.sync.dma_start(out=wqc_sb[:], in_=wqc[:])
                wkc_sb = pcw.tile([128, 2, NB, 128], bf16)
                nc.sync.dma_start(out=wkc_sb[:], in_=wkc[:])
                wvc_sb = pcw.tile([128, 2, NB, 128], bf16)
                nc.sync.dma_start(out=wvc_sb[:], in_=wvc[:])

                with tc.tile_pool(name="pcp1", bufs=2, space="PSUM") as pcp1:
                    for n in range(NB):
                        for wsb, dst in ((wqc_sb, qc_sb), (wkc_sb, kc_sb), (wvc_sb, vc_sb)):
                            ps = pcp1.tile([128, BSH], f32, tag="proj")
                            for s in range(2):
                                nc.tensor.matmul(ps[:], wsb[:, s, n, :],
                                                 hnewT_sb[:, 2 * n + s, :],
                                                 start=(s == 0), stop=(s == 1))
                            nc.scalar.copy(dst[:, n, :], ps[:])

                with tc.tile_pool(name="psS", bufs=1, space="PSUM") as psS:
                    S = psS.tile([32, NB, BSH], f32)
                    for q in range(NB):
                        pr = pctmp.tile([128, NB, BSH], bf16, tag="pr", name="pr")
                        qa = qc_sb[:, q, :]
                        qbc = bass.AP(tensor=qa.tensor, offset=qa.offset,
                                      ap=[qa.ap[0], [0, NB], qa.ap[-1]])
                        nc.vector.tensor_tensor(pr[:], qbc, kc_sb[:], OP.mult)
                        for k in range(NB):
                            nc.tensor.matmul(S[:, k, :], hq_sb[:, q, :], pr[:, k, :],
                                             start=(q == 0), stop=(q == 7))
                    nc.scalar.activation(exp_sb[:], S[:], AF.Exp,
                                         scale=float(1.0 / np.sqrt(32.0)))
                    denom = pctmp.tile([32, BSH], f32, tag="denom")
                    nc.vector.reduce_sum(denom[:], exp_sb[:].rearrange("p k b -> p b k"),
                                         axis=AX.X)
                    nc.vector.reciprocal(recip_sb[:], denom[:])

                with tc.tile_pool(name="psU", bufs=1, space="PSUM") as psU, \
                     tc.tile_pool(name="psRE", bufs=2, space="PSUM") as psRE:
                    for q in range(NB):
                        U = psU.tile([128, NB, BSH], f32, tag="U")
                        for kp in range(4):
                            nc.tensor.matmul(U[:, 2 * kp:2 * kp + 2, :], e32b_sb[:, q, :],
                                             exp_sb[:, 2 * kp:2 * kp + 2, :],
                                             start=True, stop=True)
                        Ub = pctmp.tile([128, NB, BSH], bf16, tag="Ub", name="Ub")
                        nc.scalar.copy(Ub[:], U[:])
                        prods = pctmp.tile([128, NB, BSH], bf16, tag="prods")
                        nc.vector.tensor_tensor(prods[:], Ub[:], vc_sb[:], OP.mult)
                        tr1 = pctmp.tile([128, 4, BSH], bf16, tag="tr1", name="tr1")
                        nc.vector.tensor_tensor(tr1[:], prods[:, 0:4, :], prods[:, 4:8, :], OP.add)
                        tr2 = pctmp.tile([128, 2, BSH], bf16, tag="tr2", name="tr2")
                        nc.vector.tensor_tensor(tr2[:], tr1[:, 0:2, :], tr1[:, 2:4, :], OP.add)
                        raw = pctmp.tile([128, BSH], f32, tag="raw")
                        nc.vector.tensor_tensor(raw[:], tr2[:, 0, :], tr2[:, 1, :], OP.add)
                        RE = psRE.tile([128, BSH], f32, tag="RE")
                        nc.tensor.matmul(RE[:], e32f_sb[:, q, :], recip_sb[:],
                                         start=True, stop=True)
                        nc.vector.tensor_tensor(coutb_sb[:, q, :], raw[:], RE[:], OP.mult)

                with tc.tile_pool(name="psOG", bufs=2, space="PSUM") as psOG:
                    for q in range(NB):
                        for bt in range(2):
                            csl = coutb_sb[:, q, bt * 128:(bt + 1) * 128]
                            ops_ = psOG.tile([128, BS], f32, tag="o")
                            nc.tensor.matmul(ops_[:], csl, fcw_sb[:], start=True, stop=False)
                            nc.tensor.matmul(ops_[:], ones1_sb[:], fcb_sb[:], start=False, stop=True)
                            gps_ = psOG.tile([128, BS], f32, tag="gg")
                            nc.tensor.matmul(gps_[:], csl, gw_sb[:], start=True, stop=False)
                            nc.tensor.matmul(gps_[:], ones1_sb[:], gb_sb[:], start=False, stop=True)
                            tano = pctmp.tile([128, BS], f32, tag="tano")
                            nc.scalar.activation(tano[:], ops_[:], AF.Tanh)
                            sg = pctmp.tile([128, BS], f32, tag="sgx")
                            nc.scalar.activation(sg[:], gps_[:], AF.Sigmoid)
                            hatt = pctmp.tile([128, BS], f32, tag="hatt")
                            nc.vector.tensor_tensor(hatt[:], sg[:], tano[:], OP.mult)
                            qsl = slice(q * BS, (q + 1) * BS)
                            nc.vector.tensor_tensor(hnew_sb[bt][:, qsl],
                                                    hnew_sb[bt][:, qsl], hatt[:], OP.add)
                            dh = pctmp.tile([128, BS], f32, tag="dhq", name="dhq")
                            nc.gpsimd.tensor_tensor(dh[:], hnew_sb[bt][:, qsl],
                                                    hx_sb[bt][:, qsl], OP.subtract)
                            ho = pctmp.tile([128, BS], f32, tag="hoq", name="hoq", bufs=4)
                            nc.vector.scalar_tensor_tensor(ho[:], dh[:],
                                                           mask_sb[bt][:, q:q + 1],
                                                           hx_sb[bt][:, qsl], OP.mult, OP.add)
                            nc.sync.dma_start(out=hx_out[bt * 128:(bt + 1) * 128, qsl],
                                              in_=ho[:])

    _install_bir_fix(nc)
    return nc


# ---------------------------------------------------------------------------
# Host wrapper
# ---------------------------------------------------------------------------

def kernel(inp, hx, cx, wq_inp, wk_inp, wv_inp, w_ih, w_hh, b_ih, b_hh,
           wq_c, wk_c, wv_c, fc_w, fc_b, gate_w, gate_b, step=None):
    global last_exec_time_ns, last_results

    inp = np.asarray(inp, np.float32)
    hx = np.asarray(hx, np.float32)
    cx = np.asarray(cx, np.float32)
    wq_inp = np.asarray(wq_inp, np.float32)
    wk_inp = np.asarray(wk_inp, np.float32)
    wv_inp = np.asarray(wv_inp, np.float32)
    w_ih = np.asarray(w_ih, np.float32)
    w_hh = np.asarray(w_hh, np.float32)
    b_ih = np.asarray(b_ih, np.float32)
    b_hh = np.asarray(b_hh, np.float32)
    wq_c = np.asarray(wq_c, np.float32)
    wk_c = np.asarray(wk_c, np.float32)
    wv_c = np.asarray(wv_c, np.float32)
    fc_w = np.asarray(fc_w, np.float32)
    fc_b = np.asarray(fc_b, np.float32)
    gate_w = np.asarray(gate_w, np.float32)
    gate_b = np.asarray(gate_b, np.float32)

    if "nc" not in _CACHE:
        _CACHE["nc"] = _build()
    nc = _CACHE["nc"]

    # shared (replicated) tensors
    # permute gate columns so 512-wide block j holds gate type j%4 for
    # hidden chunk j//4 (matches the device's incremental LSTM evaluation)
    perm = np.concatenate([np.arange(gt * NHID + t * 512, gt * NHID + (t + 1) * 512)
                           for t in range(4) for gt in range(4)])
    wcat = np.concatenate([w_ih.T, w_hh.T], axis=0)[:, perm]
    wcatT = np.ascontiguousarray(wcat).astype(BF16)
    biasc = (b_ih + b_hh)[perm].astype(BF16).reshape(1, GATES)
    shared = {
        "wq": np.ascontiguousarray(wq_inp.reshape(NB, 2, 128, DKI).transpose(2, 1, 0, 3)),
        "wk1": np.ascontiguousarray(wk_inp[1].reshape(8, 128, DKI).transpose(1, 0, 2)),
        "wv1": np.ascontiguousarray(wv_inp[1].reshape(8, 128, BS).transpose(1, 0, 2)),
        "wcatT": wcatT,
        "biasc": biasc,
        "wqc": np.ascontiguousarray(wq_c.astype(BF16).reshape(NB, 2, 128, 128).transpose(2, 1, 0, 3)),
        "wkc": np.ascontiguousarray(wk_c.astype(BF16).reshape(NB, 2, 128, 128).transpose(2, 1, 0, 3)),
        "wvc": np.ascontiguousarray(wv_c.astype(BF16).reshape(NB, 2, 128, 128).transpose(2, 1, 0, 3)),
        "fcw": fc_w.astype(BF16),
        "gw": gate_w.astype(BF16),
        "fcb": fc_b.astype(BF16).reshape(1, BS),
        "gb": gate_b.astype(BF16).reshape(1, BS),
    }

    in_maps = []
    for c in range(NCORES):
        rs = slice(c * BSH, (c + 1) * BSH)
        inpT = inp[rs].T.reshape(8, 128, BSH).transpose(1, 0, 2)
        hxT = hx[rs].T.reshape(16, 128, BSH).transpose(1, 0, 2)
        m = {
            "inpT": np.ascontiguousarray(inpT),
            "hxT_f": np.ascontiguousarray(hxT),
            "hxT_b": np.ascontiguousarray(hxT.astype(BF16)),
            "hx_bm": np.ascontiguousarray(hx[rs]),
            "cx_bm": np.ascontiguousarray(cx[rs]),
        }
        m.update(shared)
        in_maps.append(m)

    from concourse.bass_utils import run_bass_kernel_spmd
    trace = bool(int(os.environ.get("BASS_KTRACE", "0")))
    res = run_bass_kernel_spmd(nc, in_maps, list(range(NCORES)), trace=trace)
    last_exec_time_ns = res.exec_time_ns
    last_results = res

    hx_full = np.empty((B, NHID), np.float32)
    cx_full = np.empty((B, NHID), np.float32)
    mask_full = np.empty((B, NHID), np.float32)
    for c in range(NCORES):
        rs = slice(c * BSH, (c + 1) * BSH)
        hx_full[rs] = res.results[c]["hx_out"]
        cx_full[rs] = res.results[c]["cx_out"]
        mask_full[rs] = res.results[c]["mask_out"]
    return hx_full, cx_full, mask_full
